# revision 22
# baseline (speedup 1.0000x reference)
"""Trainium2 Bass kernel for nn_Block_6975026889363 (dense transformer block
with hypernetwork-generated weights), SPMD over 8 NeuronCores.

Strategy:
  - Data-parallel over batch (16 batches -> 2 per core).
  - The big hypernet GEMMs (t @ fW_w, 402MB of fW_w in f32) are
    column-sharded across the 8 cores (each core reads 1/8th, host-cast to
    bf16), then the generated W matrices (tiny) are AllGathered on-chip.
  - The trunk stays feature-major ([feature, token]) so weights serve as
    lhsT in natural layout and channel biases are per-partition ACT biases;
    the final projections run token-major so the residual add and output
    DMA need no transpose.
  - Softmax needs no row-max subtraction (scores empirically in [-11,-2.2]);
    scores are computed transposed so exp writes the AV operand directly and
    row sums come from a ones-vector matmul on the TensorEngine.
  - Precision: bf16 for x / LN chain / attention internals / hypernet fW;
    float32r (full speed on PE for free-dim >= 256) for qkv, c_fc, c_proj,
    c_proj_mlp matmuls and the generated weights.
"""
import sys
import types

import numpy as np
import ml_dtypes

# Provide the antenv.axon_hooks shim so trace=True (e.g. via BASS_TRACE=1)
# degrades gracefully / works instead of crashing on import.
try:
    import antenv.axon_hooks  # noqa: F401
except Exception:
    try:
        _mod = types.ModuleType("antenv.axon_hooks")
        _mod._hook = None
        _mod.set_axon_ntff_profile_hook = lambda h: setattr(_mod, "_hook", h)
        _mod.get_axon_ntff_profile_hook = lambda: _mod._hook
        sys.modules["antenv.axon_hooks"] = _mod
        import antenv
        antenv.axon_hooks = _mod
        from trn_agent_boot.trn_boot import _ntff_profile_via_ctypes
        _mod._hook = _ntff_profile_via_ctypes("/opt/axon/libaxon_pjrt.so")
    except Exception:
        pass

import concourse.bass as bass  # noqa: F401
import concourse.bacc as bacc
import concourse.mybir as mybir
import concourse.tile as tile
from concourse import bass_utils

E = 256
B = 16
S = 2048
NCORES = 8
BL = B // NCORES            # batches per core
D3E = 3 * E                 # 768
C_ATTN = E * D3E // NCORES  # fW_w column-shard size for c_attn (24576)
C_SM = E * E // NCORES      # for c_proj / c_fc / c_proj_mlp (8192)
HTILE = 2048                # hypernet fW streaming tile (free dim)
EPS = 1e-5

F32 = mybir.dt.float32
F32R = mybir.dt.float32r
BF16 = mybir.dt.bfloat16
AF = mybir.ActivationFunctionType
ALU = mybir.AluOpType
AX = mybir.AxisListType

LN_MODS = ["aln", "mln"]
TL_MODS = ["cattn", "cproj", "cfc", "cpm"]
TL_OUT = {"cattn": D3E, "cproj": E, "cfc": E, "cpm": E}
TL_COLS = {"cattn": C_ATTN, "cproj": C_SM, "cfc": C_SM, "cpm": C_SM}

_cache = {}


def _build():
    nc = bacc.Bacc("TRN2", target_bir_lowering=False, debug=False,
                   num_devices=NCORES)

    def din(name, shape, dt):
        return nc.dram_tensor(name, shape, dt, kind="ExternalInput").ap()

    ins = {}
    ins["x"] = din("x", [BL, 2, 128, S], BF16)
    ins["te_row"] = din("te_row", [1, E], F32)
    ins["te_col"] = din("te_col", [2, 128], F32)
    ins["te_col_f"] = din("te_col_f", [2, 128], F32)
    ins["ones_row_f"] = din("ones_row_f", [1, 128], F32)
    for q in LN_MODS:
        ins[f"{q}_l1w"] = din(f"{q}_l1w", [E, E], BF16)
        ins[f"{q}_l1b_col"] = din(f"{q}_l1b_col", [2, 128], F32)
        ins[f"{q}_l2wT"] = din(f"{q}_l2wT", [E, E], F32)
        ins[f"{q}_l2b_col"] = din(f"{q}_l2b_col", [2, 128], F32)
        ins[f"{q}_fww"] = din(f"{q}_fww", [E, E], F32)
        ins[f"{q}_fbw"] = din(f"{q}_fbw", [E, E], F32)
        ins[f"{q}_fwb_col"] = din(f"{q}_fwb_col", [2, 128], F32)
        ins[f"{q}_fbb_col"] = din(f"{q}_fbb_col", [2, 128], F32)
    for m in TL_MODS:
        D = TL_OUT[m]
        ins[f"{m}_l1w"] = din(f"{m}_l1w", [E, E], F32)
        ins[f"{m}_l1b_col"] = din(f"{m}_l1b_col", [2, 128], F32)
        ins[f"{m}_l2w"] = din(f"{m}_l2w", [E, E], F32)
        ins[f"{m}_l2b_col"] = din(f"{m}_l2b_col", [2, 128], F32)
        ins[f"{m}_fWs"] = din(f"{m}_fWs", [E, TL_COLS[m]], BF16)
        ins[f"{m}_fWb"] = din(f"{m}_fWb", [2, 128, D], BF16)
        ins[f"{m}_fbw"] = din(f"{m}_fbw", [E, D], F32)
    ins["cattn_fbb_qk_col"] = din("cattn_fbb_qk_col", [4, 128], F32)
    ins["cattn_fbb_v_row"] = din("cattn_fbb_v_row", [1, E], F32)
    ins["cproj_fbb_row"] = din("cproj_fbb_row", [1, E], F32)
    ins["cfc_fbb_col"] = din("cfc_fbb_col", [2, 128], F32)
    ins["cpm_fbb_row"] = din("cpm_fbb_row", [1, E], F32)

    out_d = nc.dram_tensor("out", [BL, S, E], F32, kind="ExternalOutput").ap()

    with tile.TileContext(nc) as tc:
        _emit(nc, tc, ins, out_d)
    nc.compile()
    return nc


def _emit(nc, tc, ins, out_d):
    from contextlib import ExitStack
    NT = S // 512            # 512-wide free chunks per batch (4)
    TCH = S // 128           # token 128-chunks per batch (16)

    ctx = ExitStack()
    B1, B2 = 6, 5
    con = ctx.enter_context(tc.tile_pool(name="con", bufs=1))
    pW = con
    ps_main = ctx.enter_context(tc.tile_pool(name="ps_main", bufs=2,
                                             space="PSUM"))
    dram = ctx.enter_context(tc.tile_pool(name="dram", bufs=1, space="DRAM"))

    def mmtile():
        return ps_main.tile([128, 512], F32, name="mm", tag="mm", bufs=2)

    sb = {}

    def load2(pool, name, dt, d2=E, eng=None):
        t = pool.tile([128, 2, d2], dt, name=name, tag=name)
        (eng or nc.scalar).dma_start(
            t[:], ins[name].rearrange("(kc p) m -> p kc m", p=128))
        return t

    def loadcol(pool, name, w=2, dt=F32, eng=None):
        t = pool.tile([128, w], dt, name=name, tag=name)
        (eng or nc.scalar).dma_start(t[:], ins[name].rearrange("a p -> p a"))
        return t

    def loadrow(pool, name, dt=F32):
        t = pool.tile([1, E], dt, name=name, tag=name)
        nc.scalar.dma_start(t[:], ins[name])
        return t

    # ------------- permanent constants -------------
    sb["te_row"] = con.tile([1, E], F32, name="te_row")
    nc.sync.dma_start(sb["te_row"][:], ins["te_row"])
    sb["te_col"] = loadcol(con, "te_col", dt=F32, eng=nc.sync)
    sb["te_col_f"] = loadcol(con, "te_col_f", dt=F32)
    sb["ones_row_f"] = con.tile([1, 128], F32, name="ones_row_f")
    nc.sync.dma_start(sb["ones_row_f"][:], ins["ones_row_f"])
    for q in LN_MODS:
        sb[f"{q}_l1w"] = load2(con, f"{q}_l1w", BF16)
        sb[f"{q}_l1b_col"] = loadcol(con, f"{q}_l1b_col")
    for m in TL_MODS:
        sb[f"{m}_l1b_col"] = loadcol(con, f"{m}_l1b_col")
        sb[f"{m}_l2b_col"] = loadcol(con, f"{m}_l2b_col")
        sb[f"{m}_fWb"] = con.tile([128, 2, TL_OUT[m]], BF16, name=f"{m}_fWb",
                                  tag=f"{m}_fWb")
        nc.sync.dma_start(sb[f"{m}_fWb"][:],
                          ins[f"{m}_fWb"].rearrange("a p d -> p a d"))
    sb["cattn_fbb_qk_col"] = loadcol(con, "cattn_fbb_qk_col", w=4)
    sb["cfc_fbb_col"] = loadcol(con, "cfc_fbb_col")
    sb["cattn_fbb_v_row"] = loadrow(con, "cattn_fbb_v_row", dt=F32)
    sb["cproj_fbb_row"] = loadrow(con, "cproj_fbb_row", dt=F32)
    sb["cpm_fbb_row"] = loadrow(con, "cpm_fbb_row", dt=F32)

    ones_col_bf = con.tile([128, 1], BF16, name="ones_col_bf")
    nc.vector.memset(ones_col_bf[:], 1.0)

    # ------------- early-only weights (scoped; closes before 'big' opens) ---
    _early_cm = tc.tile_pool(name="early", bufs=1)
    ep = _early_cm.__enter__()
    for m in TL_MODS:
        sb[f"{m}_l1w"] = load2(ep, f"{m}_l1w", F32, eng=nc.sync)
        sb[f"{m}_l2w"] = load2(ep, f"{m}_l2w", F32, eng=nc.sync)
    for q in LN_MODS:
        sb[f"{q}_l2wT"] = load2(ep, f"{q}_l2wT", F32)
        sb[f"{q}_l2b_col"] = loadcol(ep, f"{q}_l2b_col", dt=F32)
        sb[f"{q}_fww"] = load2(ep, f"{q}_fww", F32)
        sb[f"{q}_fbw"] = load2(ep, f"{q}_fbw", F32)
        sb[f"{q}_fwb_col"] = loadcol(ep, f"{q}_fwb_col")
        sb[f"{q}_fbb_col"] = loadcol(ep, f"{q}_fbb_col")
    for m in TL_MODS:
        sb[f"{m}_fbw"] = load2(ep, f"{m}_fbw", F32, d2=TL_OUT[m])

    # ------------- time-embed stats -> norm -------------
    st = ep.tile([1, 8], F32, name="st")
    nc.vector.reduce_sum(st[:, 0:1], sb["te_row"][:], axis=AX.X)
    nc.vector.tensor_scalar_mul(st[:, 1:2], st[:, 0:1], 1.0 / E)   # mean
    cen = ep.tile([1, E], F32, name="cen")
    nc.vector.tensor_scalar(cen[:], sb["te_row"][:], st[:, 1:2], None,
                            ALU.subtract)
    sq = ep.tile([1, E], F32, name="sq")
    nc.vector.tensor_tensor(sq[:], cen[:], cen[:], ALU.mult)
    nc.vector.reduce_sum(st[:, 2:3], sq[:], axis=AX.X)
    nc.vector.tensor_scalar(st[:, 3:4], st[:, 2:3], 1.0 / E, EPS,
                            ALU.mult, ALU.add)                      # var+eps
    # sqrt seed + one Newton step, then rstd = 1/sqrt
    nc.scalar.activation(st[:, 4:5], st[:, 3:4], AF.Sqrt)
    nc.vector.reciprocal(st[:, 5:6], st[:, 4:5])
    nc.vector.tensor_tensor(st[:, 6:7], st[:, 3:4], st[:, 5:6], ALU.mult)
    nc.vector.tensor_tensor(st[:, 6:7], st[:, 6:7], st[:, 4:5], ALU.add)
    nc.vector.tensor_scalar_mul(st[:, 6:7], st[:, 6:7], 0.5)        # sqrt(v)
    nc.vector.reciprocal(st[:, 7:8], st[:, 6:7])                    # rstd
    norm_row = ep.tile([1, E], F32, name="norm_row")
    nc.vector.tensor_scalar(norm_row[:], cen[:], st[:, 7:8], None, ALU.mult)

    # broadcast mean/rstd across partitions via K=1 matmuls, then norm_col
    mr_sb = ep.tile([128, 2], F32, name="mr_sb")
    pt = mmtile()
    nc.tensor.matmul(pt[:, 0:1], sb["ones_row_f"][:], st[:, 1:2],
                     start=True, stop=True)
    nc.tensor.matmul(pt[:, 1:2], sb["ones_row_f"][:], st[:, 7:8],
                     start=True, stop=True)
    nc.vector.tensor_copy(mr_sb[:], pt[:, 0:2])
    norm_col = ep.tile([128, 2], F32, name="norm_col")
    nc.vector.tensor_scalar(norm_col[:], sb["te_col_f"][:], mr_sb[:, 0:1],
                            None, ALU.subtract)
    nc.vector.tensor_scalar(norm_col[:], norm_col[:], mr_sb[:, 1:2],
                            None, ALU.mult)
    # norm broadcast across partitions (for Wc prep)
    norm_bc = ep.tile([128, E], F32, name="norm_bc")
    pt = mmtile()
    nc.tensor.matmul(pt[:, :E], sb["ones_row_f"][:], norm_row[:],
                     start=True, stop=True)
    nc.vector.tensor_copy(norm_bc[:], pt[:, :E])

    # ------------- t vectors (4 TL modules) -------------
    t_col, t_col_bf = {}, {}
    for m in TL_MODS:
        h_t = ep.tile([128, 2], F32, name=f"ht_{m}")
        for mm in range(2):
            pt = mmtile()
            for kc in range(2):
                nc.tensor.matmul(pt[:, 0:1],
                                 sb[f"{m}_l1w"][:, kc, 128 * mm:128 * (mm + 1)],
                                 sb["te_col"][:, kc:kc + 1],
                                 start=(kc == 0), stop=(kc == 1))
            nc.scalar.activation(h_t[:, mm:mm + 1], pt[:, 0:1], AF.Silu,
                                 bias=sb[f"{m}_l1b_col"][:, mm:mm + 1])
        tcl = con.tile([128, 2], F32, name=f"t_{m}")
        for mm in range(2):
            pt = mmtile()
            for kc in range(2):
                nc.tensor.matmul(pt[:, 0:1],
                                 sb[f"{m}_l2w"][:, kc, 128 * mm:128 * (mm + 1)],
                                 h_t[:, kc:kc + 1],
                                 start=(kc == 0), stop=(kc == 1))
            nc.vector.tensor_scalar(tcl[:, mm:mm + 1], pt[:, 0:1],
                                    sb[f"{m}_l2b_col"][:, mm:mm + 1],
                                    None, ALU.add)
        t_col[m] = tcl
        tb = con.tile([128, 2], BF16, name=f"tbf_{m}")
        nc.vector.tensor_copy(tb[:], tcl[:])
        t_col_bf[m] = tb

    # ------------- fused LN weights (W2c bf16, b2c col) -------------
    W2c, b2c = {}, {}
    for q in LN_MODS:
        wc = ep.tile([128, 2, E], F32, name=f"wc_{q}")
        for kc in range(2):
            nc.vector.tensor_tensor(wc[:, kc, :], sb[f"{q}_fww"][:, kc, :],
                                    norm_bc[:], ALU.mult)
            nc.vector.tensor_tensor(wc[:, kc, :], wc[:, kc, :],
                                    sb[f"{q}_fbw"][:, kc, :], ALU.add)
        w2 = con.tile([128, 2, E], BF16, name=f"w2c_{q}")
        for mm in range(2):
            pt = mmtile()
            for kc in range(2):
                nc.tensor.matmul(pt[:, :E],
                                 sb[f"{q}_l2wT"][:, kc, 128 * mm:128 * (mm + 1)],
                                 wc[:, kc, :], start=(kc == 0), stop=(kc == 1))
            nc.vector.tensor_copy(w2[:, mm, :], pt[:, :E])
        W2c[q] = w2
        bc = ep.tile([128, 2], F32, name=f"bc_{q}")
        nc.vector.tensor_scalar(bc[:], sb[f"{q}_fwb_col"][:], 1.0, None,
                                ALU.add)
        nc.vector.tensor_tensor(bc[:], bc[:], norm_col[:], ALU.mult)
        nc.vector.tensor_tensor(bc[:], bc[:], sb[f"{q}_fbb_col"][:], ALU.add)
        b2 = con.tile([128, 2], F32, name=f"b2c_{q}")
        for mm in range(2):
            pt = mmtile()
            for kc in range(2):
                nc.tensor.matmul(pt[:, 0:1],
                                 wc[:, kc, 128 * mm:128 * (mm + 1)],
                                 sb[f"{q}_l2b_col"][:, kc:kc + 1],
                                 start=(kc == 0), stop=(kc == 1))
            nc.vector.tensor_tensor(b2[:, mm:mm + 1], pt[:, 0:1],
                                    bc[:, mm:mm + 1], ALU.add)
        b2c[q] = b2

    # ------------- hypernet-generated biases -------------
    b_qk_col = con.tile([128, 4], F32, name="b_qk_col")
    for mm in range(4):
        pt = mmtile()
        for kc in range(2):
            nc.tensor.matmul(pt[:, 0:1],
                             sb["cattn_fbw"][:, kc, 128 * mm:128 * (mm + 1)],
                             t_col["cattn"][:, kc:kc + 1],
                             start=(kc == 0), stop=(kc == 1))
        nc.vector.tensor_tensor(b_qk_col[:, mm:mm + 1], pt[:, 0:1],
                                sb["cattn_fbb_qk_col"][:, mm:mm + 1], ALU.add)
    b_cfc_col = con.tile([128, 2], F32, name="b_cfc_col")
    for mm in range(2):
        pt = mmtile()
        for kc in range(2):
            nc.tensor.matmul(pt[:, 0:1],
                             sb["cfc_fbw"][:, kc, 128 * mm:128 * (mm + 1)],
                             t_col["cfc"][:, kc:kc + 1],
                             start=(kc == 0), stop=(kc == 1))
        nc.vector.tensor_tensor(b_cfc_col[:, mm:mm + 1], pt[:, 0:1],
                                sb["cfc_fbb_col"][:, mm:mm + 1], ALU.add)

    def brow(mod, cols, fbb_name, name):
        r = con.tile([1, E], F32, name=name)
        pt = mmtile()
        for kc in range(2):
            nc.tensor.matmul(pt[0:1, :E], t_col[mod][:, kc:kc + 1],
                             sb[f"{mod}_fbw"][:, kc, cols],
                             start=(kc == 0), stop=(kc == 1))
        nc.vector.tensor_tensor(r[:], pt[0:1, :E], sb[fbb_name][:], ALU.add)
        return r

    b_v_row = brow("cattn", slice(512, 768), "cattn_fbb_v_row", "b_v_row")
    b_cproj_row = brow("cproj", slice(0, E), "cproj_fbb_row", "b_cproj_row")
    b_cpm_row = brow("cpm", slice(0, E), "cpm_fbb_row", "b_cpm_row")
    b_fin_row = con.tile([1, E], F32, name="b_fin_row")
    nc.vector.tensor_tensor(b_fin_row[:], b_cproj_row[:], b_cpm_row[:],
                            ALU.add)
    # broadcast rows across partitions once (K=1 fp32 matmuls kept out of the
    # hot bf16 streams)
    bv_bc = con.tile([128, E], F32, name="bv_bc")
    pt = mmtile()
    nc.tensor.matmul(pt[:, :E], sb["ones_row_f"][:], b_v_row[:],
                     start=True, stop=True)
    nc.vector.tensor_copy(bv_bc[:], pt[:, :E])
    bfin_bc = con.tile([128, E], F32, name="bfin_bc")
    pt = mmtile()
    nc.tensor.matmul(pt[:, :E], sb["ones_row_f"][:], b_fin_row[:],
                     start=True, stop=True)
    nc.vector.tensor_copy(bfin_bc[:], pt[:, :E])

    # ------------- close early pool; open the big activation pool -------------
    _early_cm.__exit__(None, None, None)
    big = ctx.enter_context(tc.tile_pool(name="big", bufs=1))

    # ------------- x loads (DMA transpose, bf16) -------------
    xT = []
    for b in range(BL):
        t = big.tile([128, 2, S], BF16, name=f"xT{b}", tag="b1", bufs=B1)
        nc.sync.dma_start(t[:], ins["x"][b].rearrange("kc p s -> p kc s"))
        xT.append(t)


    # ------------- hypernet fW slices + AllGather -------------
    warm_in = dram.tile([16], F32, name="warm_in")
    warm_out = dram.tile([NCORES, 16], F32, name="warm_out",
                         addr_space="Shared")
    warm_sb = con.tile([1, 16], F32, name="warm_sb")
    nc.vector.memset(warm_sb[:], 0.0)
    nc.gpsimd.dma_start(warm_in[:], warm_sb[:])
    nc.gpsimd.collective_compute(
        "AllGather", ALU.bypass,
        replica_groups=[list(range(NCORES))],
        ins=[warm_in.opt()], outs=[warm_out.opt()])

    w_slice_cattn = dram.tile([C_ATTN], F32R, name="wsl_cattn")
    w_gather_cattn = dram.tile([NCORES, C_ATTN], F32R, name="wg_cattn",
                               addr_space="Shared")
    w_slice_rest = dram.tile([3, C_SM], F32R, name="wsl_rest")
    w_gather_rest = dram.tile([NCORES, 3, C_SM], F32R, name="wg_rest",
                              addr_space="Shared")

    with tc.tile_pool(name="ps_hy", bufs=2, space="PSUM") as ps_hy:

        def hyper_mod(m, dst_ap):
            cols = TL_COLS[m]
            fws = ins[f"{m}_fWs"].rearrange("(kc p) c -> p kc c", p=128)
            for nt in range(cols // HTILE):
                ft = big.tile([128, 2, HTILE], BF16, name="hyft", tag="b1",
                              bufs=B1)
                if m == "cattn":
                    eng = nc.sync
                else:
                    eng = nc.sync if nt % 2 == 0 else nc.scalar
                eng.dma_start(ft[:], fws[:, :, nt * HTILE:(nt + 1) * HTILE])
                for sub in range(HTILE // 1024):
                    hp = ps_hy.tile([1, 1024], F32, name="hy", tag="hy",
                                    bufs=2)
                    for nn in range(2):
                        for kc in range(2):
                            nc.tensor.matmul(
                                hp[:, nn * 512:(nn + 1) * 512],
                                t_col_bf[m][:, kc:kc + 1],
                                ft[:, kc, sub * 1024 + nn * 512:
                                   sub * 1024 + (nn + 1) * 512],
                                start=(kc == 0), stop=(kc == 1))
                    hs = con.tile([1, 1024], F32R, name="hys", tag="hys",
                                  bufs=3)
                    nc.vector.tensor_copy(hs[:], hp[:])
                    off = nt * HTILE + sub * 1024
                    nc.gpsimd.dma_start(dst_ap[off:off + 1024], hs[:])

        hyper_mod("cattn", w_slice_cattn)
        nc.gpsimd.collective_compute(
            "AllGather", ALU.bypass,
            replica_groups=[list(range(NCORES))],
            ins=[w_slice_cattn.opt()], outs=[w_gather_cattn.opt()])
        for mi, m in enumerate(["cproj", "cfc", "cpm"]):
            hyper_mod(m, w_slice_rest[mi])
        nc.gpsimd.collective_compute(
            "AllGather", ALU.bypass,
            replica_groups=[list(range(NCORES))],
            ins=[w_slice_rest.opt()], outs=[w_gather_rest.opt()])

    # ------------- aln / mln (both batches) -------------
    def temporal_ln(q, b, out_tile):
        h1 = big.tile([128, 2, S], BF16, name=f"h1_{q}{b}", tag="b1", bufs=B1)
        for mm in range(2):
            for n in range(NT):
                ns = slice(512 * n, 512 * (n + 1))
                pt = mmtile()
                for kc in range(2):
                    nc.tensor.matmul(
                        pt[:], sb[f"{q}_l1w"][:, kc, 128 * mm:128 * (mm + 1)],
                        xT[b][:, kc, ns], start=(kc == 0), stop=(kc == 1))
                nc.scalar.activation(h1[:, mm, ns], pt[:], AF.Silu,
                                     bias=sb[f"{q}_l1b_col"][:, mm:mm + 1])
        for mm in range(2):
            for n in range(NT):
                ns = slice(512 * n, 512 * (n + 1))
                pt = mmtile()
                for kc in range(2):
                    nc.tensor.matmul(
                        pt[:], W2c[q][:, kc, 128 * mm:128 * (mm + 1)],
                        h1[:, kc, ns], start=(kc == 0), stop=(kc == 1))
                nc.vector.tensor_scalar(out_tile[:, mm, ns], pt[:],
                                        b2c[q][:, mm:mm + 1], None, ALU.add)

    aln_out, mln_out = [], []
    for b in range(BL):
        t = big.tile([128, 2, S], F32R, name=f"alno{b}", tag="b2", bufs=B2)
        temporal_ln("aln", b, t)
        aln_out.append(t)
    for b in range(BL):
        t = big.tile([128, 2, S], F32R, name=f"mlno{b}", tag="b2", bufs=B2)
        temporal_ln("mln", b, t)
        mln_out.append(t)

    # ------------- assemble gathered W matrices -------------
    def assemble(name, gath_ap, D, fWb):
        w = pW.tile([128, 2, D], F32R, name=name, tag=name, bufs=1)
        for c in range(NCORES):
            nc.gpsimd.dma_start(
                w[32 * (c % 4):32 * (c % 4) + 32, c // 4, :],
                gath_ap[c].rearrange("(r o) -> r o", o=D))
        nc.vector.tensor_tensor(w[:], w[:], fWb[:], ALU.add)
        return w

    W_cattn = assemble("W_cattn", w_gather_cattn, D3E, sb["cattn_fWb"])
    W_cproj = assemble("W_cproj", w_gather_rest[:, 0], E, sb["cproj_fWb"])
    W_cfc = assemble("W_cfc", w_gather_rest[:, 1], E, sb["cfc_fWb"])
    W_cpm = assemble("W_cpm", w_gather_rest[:, 2], E, sb["cpm_fWb"])

    # ------------- qkv (Q^T,K^T feature-major; V token-major) -------------
    q_sb, k_sb, v_sb = [], [], []
    for b in range(BL):
        q = big.tile([128, 2, S], BF16, name=f"q{b}", tag="b1", bufs=B1)
        k = big.tile([128, 2, S], BF16, name=f"k{b}", tag="b1", bufs=B1)
        for mm in range(4):
            dst = q[:, mm, :] if mm < 2 else k[:, mm - 2, :]
            for n in range(NT):
                ns = slice(512 * n, 512 * (n + 1))
                pt = mmtile()
                for kc in range(2):
                    nc.tensor.matmul(
                        pt[:], W_cattn[:, kc, 128 * mm:128 * (mm + 1)],
                        aln_out[b][:, kc, ns], start=(kc == 0), stop=(kc == 1))
                nc.vector.tensor_scalar(dst[:, ns], pt[:],
                                        b_qk_col[:, mm:mm + 1], None, ALU.add)
        q_sb.append(q)
        k_sb.append(k)
        v = big.tile([128, TCH, E], BF16, name=f"v{b}", tag="b1", bufs=B1)
        for t in range(TCH):
            pt = mmtile()
            for kc in range(2):
                nc.tensor.matmul(
                    pt[:, :E], aln_out[b][:, kc, 128 * t:128 * (t + 1)],
                    W_cattn[:, kc, 512:768], start=(kc == 0), stop=(kc == 1))
            nc.vector.tensor_tensor(v[:, t, :], pt[:, :E], bv_bc[:], ALU.add)
        v_sb.append(v)

    # ------------- attention (both batches) -------------
    exp_insts = {0: [], 1: []}
    gelu_insts = {0: [], 1: []}
    attn_un = []
    rs_row = []
    h_sb = []
    with tc.tile_pool(name="ps_sc", bufs=2, space="PSUM") as ps_sc, \
         tc.tile_pool(name="ps_av", bufs=2, space="PSUM") as ps_av, \
         tc.tile_pool(name="ps_rs", bufs=1, space="PSUM") as ps_rs:

        def attention(b):
            attn_un.append(big.tile([128, 2, S], F32R, name=f"au{b}",
                                    tag="b2", bufs=B2))
            rs_row.append(con.tile([1, S], F32, name=f"rs{b}", tag="rsr",
                                   bufs=2))
            for sc in range(NT):
                s1 = slice(512 * sc, 512 * (sc + 1))
                av = [ps_av.tile([128, 512], F32, name="av", tag="av", bufs=2)
                      for _ in range(2)]
                rs = ps_rs.tile([1, 512], F32, name="rsp", tag="rsp", bufs=1)
                for s2 in range(TCH):
                    sp = ps_sc.tile([128, 512], F32, name="sc", tag="sc",
                                    bufs=3)
                    for kc in range(2):
                        nc.tensor.matmul(
                            sp[:], k_sb[b][:, kc, 128 * s2:128 * (s2 + 1)],
                            q_sb[b][:, kc, s1], start=(kc == 0),
                            stop=(kc == 1))
                    mt = con.tile([128, 512], BF16, name="mt", tag="mt",
                                  bufs=4)
                    _ei = nc.scalar.activation(mt[:], sp[:], AF.Exp,
                                               scale=1.0 / 16.0)
                    exp_insts[b].append(_ei)
                    nc.tensor.matmul(rs[:], ones_col_bf[:], mt[:],
                                     start=(s2 == 0), stop=(s2 == TCH - 1))
                    for e in range(2):
                        nc.tensor.matmul(
                            av[e][:], v_sb[b][:, s2, 128 * e:128 * (e + 1)],
                            mt[:], start=(s2 == 0), stop=(s2 == TCH - 1))
                nc.vector.tensor_copy(rs_row[b][:, s1], rs[:])
                for e in range(2):
                    nc.vector.tensor_copy(attn_un[b][:, e, s1], av[e][:])

        def cfc(b):
            h = big.tile([128, 2, S], F32R, name=f"h{b}", tag="b2", bufs=B2)
            for mm in range(2):
                for n in range(NT):
                    ns = slice(512 * n, 512 * (n + 1))
                    pt = mmtile()
                    for kc in range(2):
                        nc.tensor.matmul(
                            pt[:], W_cfc[:, kc, 128 * mm:128 * (mm + 1)],
                            mln_out[b][:, kc, ns], start=(kc == 0),
                            stop=(kc == 1))
                    _gi = nc.scalar.activation(h[:, mm, ns], pt[:], AF.Gelu,
                                               bias=b_cfc_col[:, mm:mm + 1])
                    gelu_insts[b].append(_gi)
            h_sb.append(h)

        from concourse.tile import add_dep_helper
        attention(0)
        cfc(0)
        add_dep_helper(gelu_insts[0][0].ins, exp_insts[0][-1].ins,
                       reason="ACT tables: gelu0 after exp0")
        attention(1)
        add_dep_helper(exp_insts[1][0].ins, gelu_insts[0][-1].ins,
                       reason="ACT tables: exp1 after gelu0")
        cfc(1)
        add_dep_helper(gelu_insts[1][0].ins, exp_insts[1][-1].ins,
                       reason="ACT tables: gelu1 after exp1")


    # ------------- softmax denominators -> column form -------------
    r_col = []
    for b in range(BL):
        rs_dram = dram.tile([S], F32, name=f"rsd{b}", tag=f"rsd{b}")
        nc.gpsimd.dma_start(rs_dram[:], rs_row[b][:])
        rsc = con.tile([128, TCH], F32, name=f"rsc{b}", tag="rsc", bufs=2)
        nc.gpsimd.dma_start(rsc[:], rs_dram.rearrange("(t p) -> p t", p=128))
        rc = con.tile([128, TCH], F32, name=f"rc{b}", tag="rc", bufs=2)
        nc.vector.reciprocal(rc[:], rsc[:])
        r_col.append(rc)

    # ------------- final fused projections (token-major) -------------
    for b in range(BL):
        for t in range(TCH):
            tsl = slice(128 * t, 128 * (t + 1))
            pa = mmtile()
            for kc in range(2):
                nc.tensor.matmul(pa[:, :E], attn_un[b][:, kc, tsl],
                                 W_cproj[:, kc, :],
                                 start=(kc == 0), stop=(kc == 1))
            pm = mmtile()
            for kc in range(2):
                nc.tensor.matmul(pm[:, :E], h_sb[b][:, kc, tsl],
                                 W_cpm[:, kc, :],
                                 start=(kc == 0), stop=(kc == 1))
            o1 = con.tile([128, E], F32, name="o1", tag="o1", bufs=3)
            nc.vector.tensor_tensor(o1[:], pm[:, :E], bfin_bc[:], ALU.add)
            o_sb = con.tile([128, E], F32, name="osb", tag="osb", bufs=3)
            nc.vector.scalar_tensor_tensor(
                o_sb[:], pa[:, :E], r_col[b][:, t:t + 1], o1[:],
                ALU.mult, ALU.add)
            (nc.sync if t % 2 == 0 else nc.scalar).dma_start(
                out_d[b, tsl, :], o_sb[:])

    ctx.close()


def _prep_inputs(p_aln, p_mln, p_cattn, p_cproj, p_cfc, p_cproj_mlp,
                 time_embed, x):
    f32 = np.float32
    bf16 = ml_dtypes.bfloat16
    te = np.asarray(time_embed, f32)
    com = {
        "te_row": te.reshape(1, E).copy(),
        "te_col": te.reshape(2, 128).copy(),
        "te_col_f": te.reshape(2, 128).copy(),
        "ones_row_f": np.ones((1, 128), f32),
    }
    for q, p in (("aln", p_aln), ("mln", p_mln)):
        com[f"{q}_l1w"] = np.asarray(p["lin1_w"], f32).astype(bf16)
        com[f"{q}_l1b_col"] = np.asarray(p["lin1_b"], f32).reshape(2, 128).copy()
        com[f"{q}_l2wT"] = np.asarray(p["lin2_w"], f32).T.copy()
        com[f"{q}_l2b_col"] = np.asarray(p["lin2_b"], f32).reshape(2, 128).copy()
        com[f"{q}_fww"] = np.asarray(p["fw_w"], f32)
        com[f"{q}_fbw"] = np.asarray(p["fb_w"], f32)
        com[f"{q}_fwb_col"] = np.asarray(p["fw_b"], f32).reshape(2, 128).copy()
        com[f"{q}_fbb_col"] = np.asarray(p["fb_b"], f32).reshape(2, 128).copy()
    tl = (("cattn", p_cattn), ("cproj", p_cproj), ("cfc", p_cfc),
          ("cpm", p_cproj_mlp))
    fWs_bf = {}
    for m, p in tl:
        D = TL_OUT[m]
        com[f"{m}_l1w"] = np.asarray(p["lin1_w"], f32)
        com[f"{m}_l1b_col"] = np.asarray(p["lin1_b"], f32).reshape(2, 128).copy()
        com[f"{m}_l2w"] = np.asarray(p["lin2_w"], f32)
        com[f"{m}_l2b_col"] = np.asarray(p["lin2_b"], f32).reshape(2, 128).copy()
        com[f"{m}_fWb"] = np.asarray(p["fW_b"], f32).reshape(E, D) \
            .reshape(2, 128, D).astype(bf16)
        com[f"{m}_fbw"] = np.asarray(p["fb_w"], f32)
        fWs_bf[m] = np.asarray(p["fW_w"], f32).astype(bf16)
    fbb_cattn = np.asarray(p_cattn["fb_b"], f32)
    com["cattn_fbb_qk_col"] = fbb_cattn[:512].reshape(4, 128).copy()
    com["cattn_fbb_v_row"] = fbb_cattn[512:].reshape(1, E).copy()
    com["cproj_fbb_row"] = np.asarray(p_cproj["fb_b"], f32).reshape(1, E).copy()
    com["cfc_fbb_col"] = np.asarray(p_cfc["fb_b"], f32).reshape(2, 128).copy()
    com["cpm_fbb_row"] = np.asarray(p_cproj_mlp["fb_b"], f32).reshape(1, E).copy()

    x_bf = np.asarray(x, f32).astype(bf16)
    in_maps = []
    for c in range(NCORES):
        im = dict(com)
        xs = x_bf[c * BL:(c + 1) * BL]          # [BL, S, E]
        im["x"] = np.ascontiguousarray(
            xs.reshape(BL, S, 2, 128).transpose(0, 2, 3, 1))
        im["cattn_fWs"] = np.ascontiguousarray(
            fWs_bf["cattn"][:, c * C_ATTN:(c + 1) * C_ATTN])
        for m in ("cproj", "cfc", "cpm"):
            im[f"{m}_fWs"] = np.ascontiguousarray(
                fWs_bf[m][:, c * C_SM:(c + 1) * C_SM])
        in_maps.append(im)
    return in_maps


def kernel(p_aln, p_mln, p_cattn, p_cproj, p_cfc, p_cproj_mlp,
           time_embed, x):
    if "nc" not in _cache:
        _cache["nc"] = _build()
    nc = _cache["nc"]
    in_maps = _prep_inputs(p_aln, p_mln, p_cattn, p_cproj, p_cfc,
                           p_cproj_mlp, time_embed, x)
    res = bass_utils.run_bass_kernel_spmd(
        nc, in_maps, core_ids=list(range(NCORES)))
    out = np.concatenate([res.results[c]["out"] for c in range(NCORES)],
                         axis=0)
    return out.astype(np.float32)


# revision 25
# speedup vs baseline: 1.0580x; 1.0580x over previous
"""Trainium2 Bass kernel for nn_Block_6975026889363 (dense transformer block
with hypernetwork-generated weights), SPMD over 8 NeuronCores.

Strategy:
  - Data-parallel over batch (16 batches -> 2 per core).
  - The big hypernet GEMMs (t @ fW_w, 402MB of fW_w in f32) are
    column-sharded across the 8 cores (each core reads 1/8th, host-cast to
    bf16), then the generated W matrices (tiny) are AllGathered on-chip.
  - The trunk stays feature-major ([feature, token]) so weights serve as
    lhsT in natural layout and channel biases are per-partition ACT biases;
    the final projections run token-major so the residual add and output
    DMA need no transpose.
  - Softmax needs no row-max subtraction (scores empirically in [-11,-2.2]);
    scores are computed transposed so exp writes the AV operand directly and
    row sums come from a ones-vector matmul on the TensorEngine.
  - Precision: bf16 for x / LN chain / attention internals / hypernet fW;
    float32r (full speed on PE for free-dim >= 256) for qkv, c_fc, c_proj,
    c_proj_mlp matmuls and the generated weights.
"""
import sys
import types

import numpy as np
import ml_dtypes

# Provide the antenv.axon_hooks shim so trace=True (e.g. via BASS_TRACE=1)
# degrades gracefully / works instead of crashing on import.
try:
    import antenv.axon_hooks  # noqa: F401
except Exception:
    try:
        _mod = types.ModuleType("antenv.axon_hooks")
        _mod._hook = None
        _mod.set_axon_ntff_profile_hook = lambda h: setattr(_mod, "_hook", h)
        _mod.get_axon_ntff_profile_hook = lambda: _mod._hook
        sys.modules["antenv.axon_hooks"] = _mod
        import antenv
        antenv.axon_hooks = _mod
        from trn_agent_boot.trn_boot import _ntff_profile_via_ctypes
        _mod._hook = _ntff_profile_via_ctypes("/opt/axon/libaxon_pjrt.so")
    except Exception:
        pass

import concourse.bass as bass  # noqa: F401
import concourse.bacc as bacc
import concourse.mybir as mybir
import concourse.tile as tile
from concourse import bass_utils

E = 256
B = 16
S = 2048
NCORES = 8
BL = B // NCORES            # batches per core
D3E = 3 * E                 # 768
C_ATTN = E * D3E // NCORES  # fW_w column-shard size for c_attn (24576)
C_SM = E * E // NCORES      # for c_proj / c_fc / c_proj_mlp (8192)
HTILE = 2048                # hypernet fW streaming tile (free dim)
EPS = 1e-5

F32 = mybir.dt.float32
F32R = mybir.dt.float32r
BF16 = mybir.dt.bfloat16
AF = mybir.ActivationFunctionType
ALU = mybir.AluOpType
AX = mybir.AxisListType

LN_MODS = ["aln", "mln"]
TL_MODS = ["cattn", "cproj", "cfc", "cpm"]
TL_OUT = {"cattn": D3E, "cproj": E, "cfc": E, "cpm": E}
TL_COLS = {"cattn": C_ATTN, "cproj": C_SM, "cfc": C_SM, "cpm": C_SM}

_cache = {}


def _build():
    nc = bacc.Bacc("TRN2", target_bir_lowering=False, debug=False,
                   num_devices=NCORES)

    def din(name, shape, dt):
        return nc.dram_tensor(name, shape, dt, kind="ExternalInput").ap()

    ins = {}
    ins["x"] = din("x", [BL, 2, 128, S], BF16)
    ins["te_row"] = din("te_row", [1, E], F32)
    ins["te_col"] = din("te_col", [2, 128], F32)
    ins["te_col_f"] = din("te_col_f", [2, 128], F32)
    ins["ones_row_f"] = din("ones_row_f", [1, 128], F32)
    for q in LN_MODS:
        ins[f"{q}_l1w"] = din(f"{q}_l1w", [E, E], BF16)
        ins[f"{q}_l1b_col"] = din(f"{q}_l1b_col", [2, 128], F32)
        ins[f"{q}_l2wT"] = din(f"{q}_l2wT", [E, E], F32)
        ins[f"{q}_l2b_col"] = din(f"{q}_l2b_col", [2, 128], F32)
        ins[f"{q}_fww"] = din(f"{q}_fww", [E, E], F32)
        ins[f"{q}_fbw"] = din(f"{q}_fbw", [E, E], F32)
        ins[f"{q}_fwb_col"] = din(f"{q}_fwb_col", [2, 128], F32)
        ins[f"{q}_fbb_col"] = din(f"{q}_fbb_col", [2, 128], F32)
    for m in TL_MODS:
        D = TL_OUT[m]
        ins[f"{m}_l1w"] = din(f"{m}_l1w", [E, E], F32)
        ins[f"{m}_l1b_col"] = din(f"{m}_l1b_col", [2, 128], F32)
        ins[f"{m}_l2w"] = din(f"{m}_l2w", [E, E], F32)
        ins[f"{m}_l2b_col"] = din(f"{m}_l2b_col", [2, 128], F32)
        ins[f"{m}_fWs"] = din(f"{m}_fWs", [E, TL_COLS[m]], BF16)
        ins[f"{m}_fWb"] = din(f"{m}_fWb", [2, 128, D], BF16)
        ins[f"{m}_fbw"] = din(f"{m}_fbw", [E, D], F32)
    ins["cattn_fbb_qk_col"] = din("cattn_fbb_qk_col", [4, 128], F32)
    ins["cattn_fbb_v_row"] = din("cattn_fbb_v_row", [1, E], F32)
    ins["cproj_fbb_row"] = din("cproj_fbb_row", [1, E], F32)
    ins["cfc_fbb_col"] = din("cfc_fbb_col", [2, 128], F32)
    ins["cpm_fbb_row"] = din("cpm_fbb_row", [1, E], F32)

    out_d = nc.dram_tensor("out", [BL, S, E], F32, kind="ExternalOutput").ap()

    with tile.TileContext(nc) as tc:
        _emit(nc, tc, ins, out_d)
    nc.compile()
    return nc


def _emit(nc, tc, ins, out_d):
    from contextlib import ExitStack
    NT = S // 512            # 512-wide free chunks per batch (4)
    TCH = S // 128           # token 128-chunks per batch (16)

    ctx = ExitStack()
    B1, B2 = 6, 5
    con = ctx.enter_context(tc.tile_pool(name="con", bufs=1))
    pW = con
    ps_main = ctx.enter_context(tc.tile_pool(name="ps_main", bufs=2,
                                             space="PSUM"))
    dram = ctx.enter_context(tc.tile_pool(name="dram", bufs=1, space="DRAM"))

    def mmtile():
        return ps_main.tile([128, 512], F32, name="mm", tag="mm", bufs=2)

    sb = {}

    def load2(pool, name, dt, d2=E, eng=None):
        t = pool.tile([128, 2, d2], dt, name=name, tag=name)
        (eng or nc.scalar).dma_start(
            t[:], ins[name].rearrange("(kc p) m -> p kc m", p=128))
        return t

    def loadcol(pool, name, w=2, dt=F32, eng=None):
        t = pool.tile([128, w], dt, name=name, tag=name)
        (eng or nc.scalar).dma_start(t[:], ins[name].rearrange("a p -> p a"))
        return t

    def loadrow(pool, name, dt=F32):
        t = pool.tile([1, E], dt, name=name, tag=name)
        nc.scalar.dma_start(t[:], ins[name])
        return t

    # ------------- permanent constants -------------
    sb["te_row"] = con.tile([1, E], F32, name="te_row")
    nc.sync.dma_start(sb["te_row"][:], ins["te_row"])
    sb["te_col"] = loadcol(con, "te_col", dt=F32, eng=nc.sync)
    sb["te_col_f"] = loadcol(con, "te_col_f", dt=F32)
    sb["ones_row_f"] = con.tile([1, 128], F32, name="ones_row_f")
    nc.sync.dma_start(sb["ones_row_f"][:], ins["ones_row_f"])
    for q in LN_MODS:
        sb[f"{q}_l1w"] = load2(con, f"{q}_l1w", BF16)
        sb[f"{q}_l1b_col"] = loadcol(con, f"{q}_l1b_col")
    for m in TL_MODS:
        sb[f"{m}_l1b_col"] = loadcol(con, f"{m}_l1b_col")
        sb[f"{m}_l2b_col"] = loadcol(con, f"{m}_l2b_col")
        sb[f"{m}_fWb"] = con.tile([128, 2, TL_OUT[m]], BF16, name=f"{m}_fWb",
                                  tag=f"{m}_fWb")
        nc.sync.dma_start(sb[f"{m}_fWb"][:],
                          ins[f"{m}_fWb"].rearrange("a p d -> p a d"))
    sb["cattn_fbb_qk_col"] = loadcol(con, "cattn_fbb_qk_col", w=4)
    sb["cfc_fbb_col"] = loadcol(con, "cfc_fbb_col")
    sb["cattn_fbb_v_row"] = loadrow(con, "cattn_fbb_v_row", dt=F32)
    sb["cproj_fbb_row"] = loadrow(con, "cproj_fbb_row", dt=F32)
    sb["cpm_fbb_row"] = loadrow(con, "cpm_fbb_row", dt=F32)

    ones_col_bf = con.tile([128, 1], BF16, name="ones_col_bf")
    nc.vector.memset(ones_col_bf[:], 1.0)

    # ------------- early-only weights (scoped; closes before 'big' opens) ---
    _early_cm = tc.tile_pool(name="early", bufs=1)
    ep = _early_cm.__enter__()
    for m in TL_MODS:
        sb[f"{m}_l1w"] = load2(ep, f"{m}_l1w", F32, eng=nc.sync)
        sb[f"{m}_l2w"] = load2(ep, f"{m}_l2w", F32, eng=nc.sync)
    for q in LN_MODS:
        sb[f"{q}_l2wT"] = load2(ep, f"{q}_l2wT", F32)
        sb[f"{q}_l2b_col"] = loadcol(ep, f"{q}_l2b_col", dt=F32)
        sb[f"{q}_fww"] = load2(ep, f"{q}_fww", F32)
        sb[f"{q}_fbw"] = load2(ep, f"{q}_fbw", F32)
        sb[f"{q}_fwb_col"] = loadcol(ep, f"{q}_fwb_col")
        sb[f"{q}_fbb_col"] = loadcol(ep, f"{q}_fbb_col")
    for m in TL_MODS:
        sb[f"{m}_fbw"] = load2(ep, f"{m}_fbw", F32, d2=TL_OUT[m])

    # ------------- time-embed stats -> norm -------------
    st = ep.tile([1, 8], F32, name="st")
    nc.vector.reduce_sum(st[:, 0:1], sb["te_row"][:], axis=AX.X)
    nc.vector.tensor_scalar_mul(st[:, 1:2], st[:, 0:1], 1.0 / E)   # mean
    cen = ep.tile([1, E], F32, name="cen")
    nc.vector.tensor_scalar(cen[:], sb["te_row"][:], st[:, 1:2], None,
                            ALU.subtract)
    sq = ep.tile([1, E], F32, name="sq")
    nc.vector.tensor_tensor(sq[:], cen[:], cen[:], ALU.mult)
    nc.vector.reduce_sum(st[:, 2:3], sq[:], axis=AX.X)
    nc.vector.tensor_scalar(st[:, 3:4], st[:, 2:3], 1.0 / E, EPS,
                            ALU.mult, ALU.add)                      # var+eps
    # sqrt seed + one Newton step, then rstd = 1/sqrt
    nc.scalar.activation(st[:, 4:5], st[:, 3:4], AF.Sqrt)
    nc.vector.reciprocal(st[:, 5:6], st[:, 4:5])
    nc.vector.tensor_tensor(st[:, 6:7], st[:, 3:4], st[:, 5:6], ALU.mult)
    nc.vector.tensor_tensor(st[:, 6:7], st[:, 6:7], st[:, 4:5], ALU.add)
    nc.vector.tensor_scalar_mul(st[:, 6:7], st[:, 6:7], 0.5)        # sqrt(v)
    nc.vector.reciprocal(st[:, 7:8], st[:, 6:7])                    # rstd
    norm_row = ep.tile([1, E], F32, name="norm_row")
    nc.vector.tensor_scalar(norm_row[:], cen[:], st[:, 7:8], None, ALU.mult)

    # broadcast mean/rstd across partitions via K=1 matmuls, then norm_col
    mr_sb = ep.tile([128, 2], F32, name="mr_sb")
    pt = mmtile()
    nc.tensor.matmul(pt[:, 0:1], sb["ones_row_f"][:], st[:, 1:2],
                     start=True, stop=True)
    nc.tensor.matmul(pt[:, 1:2], sb["ones_row_f"][:], st[:, 7:8],
                     start=True, stop=True)
    nc.vector.tensor_copy(mr_sb[:], pt[:, 0:2])
    norm_col = ep.tile([128, 2], F32, name="norm_col")
    nc.vector.tensor_scalar(norm_col[:], sb["te_col_f"][:], mr_sb[:, 0:1],
                            None, ALU.subtract)
    nc.vector.tensor_scalar(norm_col[:], norm_col[:], mr_sb[:, 1:2],
                            None, ALU.mult)
    # norm broadcast across partitions (for Wc prep)
    norm_bc = ep.tile([128, E], F32, name="norm_bc")
    pt = mmtile()
    nc.tensor.matmul(pt[:, :E], sb["ones_row_f"][:], norm_row[:],
                     start=True, stop=True)
    nc.vector.tensor_copy(norm_bc[:], pt[:, :E])

    # ------------- t vectors (4 TL modules) -------------
    t_col, t_col_bf = {}, {}
    for m in TL_MODS:
        h_t = ep.tile([128, 2], F32, name=f"ht_{m}")
        for mm in range(2):
            pt = mmtile()
            for kc in range(2):
                nc.tensor.matmul(pt[:, 0:1],
                                 sb[f"{m}_l1w"][:, kc, 128 * mm:128 * (mm + 1)],
                                 sb["te_col"][:, kc:kc + 1],
                                 start=(kc == 0), stop=(kc == 1))
            nc.scalar.activation(h_t[:, mm:mm + 1], pt[:, 0:1], AF.Silu,
                                 bias=sb[f"{m}_l1b_col"][:, mm:mm + 1])
        tcl = con.tile([128, 2], F32, name=f"t_{m}")
        for mm in range(2):
            pt = mmtile()
            for kc in range(2):
                nc.tensor.matmul(pt[:, 0:1],
                                 sb[f"{m}_l2w"][:, kc, 128 * mm:128 * (mm + 1)],
                                 h_t[:, kc:kc + 1],
                                 start=(kc == 0), stop=(kc == 1))
            nc.vector.tensor_scalar(tcl[:, mm:mm + 1], pt[:, 0:1],
                                    sb[f"{m}_l2b_col"][:, mm:mm + 1],
                                    None, ALU.add)
        t_col[m] = tcl
        tb = con.tile([128, 2], BF16, name=f"tbf_{m}")
        nc.vector.tensor_copy(tb[:], tcl[:])
        t_col_bf[m] = tb

    # ------------- fused LN weights (W2c bf16, b2c col) -------------
    W2c, b2c = {}, {}
    for q in LN_MODS:
        wc = ep.tile([128, 2, E], F32, name=f"wc_{q}")
        for kc in range(2):
            nc.vector.tensor_tensor(wc[:, kc, :], sb[f"{q}_fww"][:, kc, :],
                                    norm_bc[:], ALU.mult)
            nc.vector.tensor_tensor(wc[:, kc, :], wc[:, kc, :],
                                    sb[f"{q}_fbw"][:, kc, :], ALU.add)
        w2 = con.tile([128, 2, E], BF16, name=f"w2c_{q}")
        for mm in range(2):
            pt = mmtile()
            for kc in range(2):
                nc.tensor.matmul(pt[:, :E],
                                 sb[f"{q}_l2wT"][:, kc, 128 * mm:128 * (mm + 1)],
                                 wc[:, kc, :], start=(kc == 0), stop=(kc == 1))
            nc.vector.tensor_copy(w2[:, mm, :], pt[:, :E])
        W2c[q] = w2
        bc = ep.tile([128, 2], F32, name=f"bc_{q}")
        nc.vector.tensor_scalar(bc[:], sb[f"{q}_fwb_col"][:], 1.0, None,
                                ALU.add)
        nc.vector.tensor_tensor(bc[:], bc[:], norm_col[:], ALU.mult)
        nc.vector.tensor_tensor(bc[:], bc[:], sb[f"{q}_fbb_col"][:], ALU.add)
        b2 = con.tile([128, 2], F32, name=f"b2c_{q}")
        for mm in range(2):
            pt = mmtile()
            for kc in range(2):
                nc.tensor.matmul(pt[:, 0:1],
                                 wc[:, kc, 128 * mm:128 * (mm + 1)],
                                 sb[f"{q}_l2b_col"][:, kc:kc + 1],
                                 start=(kc == 0), stop=(kc == 1))
            nc.vector.tensor_tensor(b2[:, mm:mm + 1], pt[:, 0:1],
                                    bc[:, mm:mm + 1], ALU.add)
        b2c[q] = b2

    # ------------- hypernet-generated biases -------------
    b_qk_col = con.tile([128, 4], F32, name="b_qk_col")
    for mm in range(4):
        pt = mmtile()
        for kc in range(2):
            nc.tensor.matmul(pt[:, 0:1],
                             sb["cattn_fbw"][:, kc, 128 * mm:128 * (mm + 1)],
                             t_col["cattn"][:, kc:kc + 1],
                             start=(kc == 0), stop=(kc == 1))
        nc.vector.tensor_tensor(b_qk_col[:, mm:mm + 1], pt[:, 0:1],
                                sb["cattn_fbb_qk_col"][:, mm:mm + 1], ALU.add)
    b_cfc_col = con.tile([128, 2], F32, name="b_cfc_col")
    for mm in range(2):
        pt = mmtile()
        for kc in range(2):
            nc.tensor.matmul(pt[:, 0:1],
                             sb["cfc_fbw"][:, kc, 128 * mm:128 * (mm + 1)],
                             t_col["cfc"][:, kc:kc + 1],
                             start=(kc == 0), stop=(kc == 1))
        nc.vector.tensor_tensor(b_cfc_col[:, mm:mm + 1], pt[:, 0:1],
                                sb["cfc_fbb_col"][:, mm:mm + 1], ALU.add)

    def brow(mod, cols, fbb_name, name):
        r = con.tile([1, E], F32, name=name)
        pt = mmtile()
        for kc in range(2):
            nc.tensor.matmul(pt[0:1, :E], t_col[mod][:, kc:kc + 1],
                             sb[f"{mod}_fbw"][:, kc, cols],
                             start=(kc == 0), stop=(kc == 1))
        nc.vector.tensor_tensor(r[:], pt[0:1, :E], sb[fbb_name][:], ALU.add)
        return r

    b_v_row = brow("cattn", slice(512, 768), "cattn_fbb_v_row", "b_v_row")
    b_cproj_row = brow("cproj", slice(0, E), "cproj_fbb_row", "b_cproj_row")
    b_cpm_row = brow("cpm", slice(0, E), "cpm_fbb_row", "b_cpm_row")
    b_fin_row = con.tile([1, E], F32, name="b_fin_row")
    nc.vector.tensor_tensor(b_fin_row[:], b_cproj_row[:], b_cpm_row[:],
                            ALU.add)
    # broadcast rows across partitions once (K=1 fp32 matmuls kept out of the
    # hot bf16 streams)
    bv_bc = con.tile([128, E], F32, name="bv_bc")
    pt = mmtile()
    nc.tensor.matmul(pt[:, :E], sb["ones_row_f"][:], b_v_row[:],
                     start=True, stop=True)
    nc.vector.tensor_copy(bv_bc[:], pt[:, :E])
    bfin_bc = con.tile([128, E], F32, name="bfin_bc")
    pt = mmtile()
    nc.tensor.matmul(pt[:, :E], sb["ones_row_f"][:], b_fin_row[:],
                     start=True, stop=True)
    nc.vector.tensor_copy(bfin_bc[:], pt[:, :E])

    # ------------- close early pool; open the big activation pool -------------
    _early_cm.__exit__(None, None, None)
    big = ctx.enter_context(tc.tile_pool(name="big", bufs=1))

    # ------------- x loads (DMA transpose, bf16) -------------
    xT = []
    for b in range(BL):
        t = big.tile([128, 2, S], BF16, name=f"xT{b}", tag="b1", bufs=B1)
        nc.sync.dma_start(t[:], ins["x"][b].rearrange("kc p s -> p kc s"))
        xT.append(t)


    # ------------- hypernet fW slices + AllGather -------------
    warm_in = dram.tile([16], F32, name="warm_in")
    warm_out = dram.tile([NCORES, 16], F32, name="warm_out",
                         addr_space="Shared")
    warm_sb = con.tile([1, 16], F32, name="warm_sb")
    nc.vector.memset(warm_sb[:], 0.0)
    nc.gpsimd.dma_start(warm_in[:], warm_sb[:])
    nc.gpsimd.collective_compute(
        "AllGather", ALU.bypass,
        replica_groups=[list(range(NCORES))],
        ins=[warm_in.opt()], outs=[warm_out.opt()])

    w_slice_cattn = dram.tile([C_ATTN], F32R, name="wsl_cattn")
    w_gather_cattn = dram.tile([NCORES, C_ATTN], F32R, name="wg_cattn",
                               addr_space="Shared")
    w_slice_rest = dram.tile([3, C_SM], F32R, name="wsl_rest")
    w_gather_rest = dram.tile([NCORES, 3, C_SM], F32R, name="wg_rest",
                              addr_space="Shared")

    with tc.tile_pool(name="ps_hy", bufs=2, space="PSUM") as ps_hy:

        def hyper_mod(m, dst_ap):
            cols = TL_COLS[m]
            fws = ins[f"{m}_fWs"].rearrange("(kc p) c -> p kc c", p=128)
            for nt in range(cols // HTILE):
                ft = big.tile([128, 2, HTILE], BF16, name="hyft", tag="b1",
                              bufs=B1)
                if m == "cattn":
                    eng = nc.sync
                else:
                    eng = nc.sync if nt % 2 == 0 else nc.scalar
                eng.dma_start(ft[:], fws[:, :, nt * HTILE:(nt + 1) * HTILE])
                # 4 col-tiled M=1 matmuls land the four 512-col sub-chunks on
                # partitions 0/32/64/96 of one PSUM bank -> one 4-row evac.
                hp = ps_hy.tile([128, 512], F32, name="hy", tag="hy", bufs=3)
                for j in range(4):
                    for kc in range(2):
                        nc.tensor.matmul(
                            hp[32 * j:32 * j + 1, :],
                            t_col_bf[m][:, kc:kc + 1],
                            ft[:, kc, 512 * j:512 * (j + 1)],
                            start=(kc == 0), stop=(kc == 1),
                            tile_position=(0, 32 * j))
                hs = con.tile([128, 512], F32R, name="hys", tag="hys",
                              bufs=4)
                for j in range(4):
                    srow = hp[32 * j:32 * j + 1, :]
                    drow = hs[32 * j:32 * j + 1, :]
                    if (nt + j) % 2 == 0:
                        nc.vector.tensor_copy(drow, srow)
                    else:
                        nc.scalar.activation(drow, srow, AF.Copy)
                off = nt * HTILE
                for j in range(4):
                    nc.gpsimd.dma_start(dst_ap[off + 512 * j:off + 512 * (j + 1)],
                                        hs[32 * j:32 * j + 1, :])

        hyper_mod("cattn", w_slice_cattn)
        nc.gpsimd.collective_compute(
            "AllGather", ALU.bypass,
            replica_groups=[list(range(NCORES))],
            ins=[w_slice_cattn.opt()], outs=[w_gather_cattn.opt()])
        for mi, m in enumerate(["cproj", "cfc", "cpm"]):
            hyper_mod(m, w_slice_rest[mi])
        nc.gpsimd.collective_compute(
            "AllGather", ALU.bypass,
            replica_groups=[list(range(NCORES))],
            ins=[w_slice_rest.opt()], outs=[w_gather_rest.opt()])

    # ------------- aln / mln (both batches) -------------
    def temporal_ln(q, b, out_tile):
        h1 = big.tile([128, 2, S], BF16, name=f"h1_{q}{b}", tag="b1", bufs=B1)
        for mm in range(2):
            for n in range(NT):
                ns = slice(512 * n, 512 * (n + 1))
                pt = mmtile()
                for kc in range(2):
                    nc.tensor.matmul(
                        pt[:], sb[f"{q}_l1w"][:, kc, 128 * mm:128 * (mm + 1)],
                        xT[b][:, kc, ns], start=(kc == 0), stop=(kc == 1))
                nc.scalar.activation(h1[:, mm, ns], pt[:], AF.Silu,
                                     bias=sb[f"{q}_l1b_col"][:, mm:mm + 1])
        for mm in range(2):
            for n in range(NT):
                ns = slice(512 * n, 512 * (n + 1))
                pt = mmtile()
                for kc in range(2):
                    nc.tensor.matmul(
                        pt[:], W2c[q][:, kc, 128 * mm:128 * (mm + 1)],
                        h1[:, kc, ns], start=(kc == 0), stop=(kc == 1))
                nc.vector.tensor_scalar(out_tile[:, mm, ns], pt[:],
                                        b2c[q][:, mm:mm + 1], None, ALU.add)

    aln_out, mln_out = [], []
    for b in range(BL):
        t = big.tile([128, 2, S], F32R, name=f"alno{b}", tag="b2", bufs=B2)
        temporal_ln("aln", b, t)
        aln_out.append(t)
    for b in range(BL):
        t = big.tile([128, 2, S], F32R, name=f"mlno{b}", tag="b2", bufs=B2)
        temporal_ln("mln", b, t)
        mln_out.append(t)

    # ------------- assemble gathered W matrices -------------
    def assemble(name, gath_ap, D, fWb):
        w = pW.tile([128, 2, D], F32R, name=name, tag=name, bufs=1)
        for c in range(NCORES):
            nc.gpsimd.dma_start(
                w[32 * (c % 4):32 * (c % 4) + 32, c // 4, :],
                gath_ap[c].rearrange("(r o) -> r o", o=D))
        nc.vector.tensor_tensor(w[:], w[:], fWb[:], ALU.add)
        return w

    W_cattn = assemble("W_cattn", w_gather_cattn, D3E, sb["cattn_fWb"])
    W_cproj = assemble("W_cproj", w_gather_rest[:, 0], E, sb["cproj_fWb"])
    W_cfc = assemble("W_cfc", w_gather_rest[:, 1], E, sb["cfc_fWb"])
    W_cpm = assemble("W_cpm", w_gather_rest[:, 2], E, sb["cpm_fWb"])

    # ------------- qkv (Q^T,K^T feature-major; V token-major) -------------
    q_sb, k_sb, v_sb = [], [], []
    for b in range(BL):
        q = big.tile([128, 2, S], BF16, name=f"q{b}", tag="b1", bufs=B1)
        k = big.tile([128, 2, S], BF16, name=f"k{b}", tag="b1", bufs=B1)
        for mm in range(4):
            dst = q[:, mm, :] if mm < 2 else k[:, mm - 2, :]
            for n in range(NT):
                ns = slice(512 * n, 512 * (n + 1))
                pt = mmtile()
                for kc in range(2):
                    nc.tensor.matmul(
                        pt[:], W_cattn[:, kc, 128 * mm:128 * (mm + 1)],
                        aln_out[b][:, kc, ns], start=(kc == 0), stop=(kc == 1))
                nc.vector.tensor_scalar(dst[:, ns], pt[:],
                                        b_qk_col[:, mm:mm + 1], None, ALU.add)
        q_sb.append(q)
        k_sb.append(k)
        v = big.tile([128, TCH, E], BF16, name=f"v{b}", tag="b1", bufs=B1)
        for t in range(TCH):
            pt = mmtile()
            for kc in range(2):
                nc.tensor.matmul(
                    pt[:, :E], aln_out[b][:, kc, 128 * t:128 * (t + 1)],
                    W_cattn[:, kc, 512:768], start=(kc == 0), stop=(kc == 1))
            nc.vector.tensor_tensor(v[:, t, :], pt[:, :E], bv_bc[:], ALU.add)
        v_sb.append(v)

    # ------------- attention (both batches) -------------
    exp_insts = {0: [], 1: []}
    gelu_insts = {0: [], 1: []}
    attn_un = []
    rs_row = []
    h_sb = []
    with tc.tile_pool(name="ps_sc", bufs=2, space="PSUM") as ps_sc, \
         tc.tile_pool(name="ps_av", bufs=2, space="PSUM") as ps_av, \
         tc.tile_pool(name="ps_rs", bufs=1, space="PSUM") as ps_rs:

        def attention(b):
            attn_un.append(big.tile([128, 2, S], F32R, name=f"au{b}",
                                    tag="b2", bufs=B2))
            rs_row.append(con.tile([1, S], F32, name=f"rs{b}", tag="rsr",
                                   bufs=2))
            for sc in range(NT):
                s1 = slice(512 * sc, 512 * (sc + 1))
                av = [ps_av.tile([128, 512], F32, name="av", tag="av", bufs=2)
                      for _ in range(2)]
                rs = ps_rs.tile([1, 512], F32, name="rsp", tag="rsp", bufs=1)
                for s2 in range(TCH):
                    sp = ps_sc.tile([128, 512], F32, name="sc", tag="sc",
                                    bufs=3)
                    for kc in range(2):
                        nc.tensor.matmul(
                            sp[:], k_sb[b][:, kc, 128 * s2:128 * (s2 + 1)],
                            q_sb[b][:, kc, s1], start=(kc == 0),
                            stop=(kc == 1))
                    mt = con.tile([128, 512], BF16, name="mt", tag="mt",
                                  bufs=4)
                    _ei = nc.scalar.activation(mt[:], sp[:], AF.Exp,
                                               scale=1.0 / 16.0)
                    exp_insts[b].append(_ei)
                    nc.tensor.matmul(rs[:], ones_col_bf[:], mt[:],
                                     start=(s2 == 0), stop=(s2 == TCH - 1))
                    for e in range(2):
                        nc.tensor.matmul(
                            av[e][:], v_sb[b][:, s2, 128 * e:128 * (e + 1)],
                            mt[:], start=(s2 == 0), stop=(s2 == TCH - 1))
                nc.vector.tensor_copy(rs_row[b][:, s1], rs[:])
                for e in range(2):
                    nc.vector.tensor_copy(attn_un[b][:, e, s1], av[e][:])

        def cfc(b):
            h = big.tile([128, 2, S], F32R, name=f"h{b}", tag="b2", bufs=B2)
            for mm in range(2):
                for n in range(NT):
                    ns = slice(512 * n, 512 * (n + 1))
                    pt = mmtile()
                    for kc in range(2):
                        nc.tensor.matmul(
                            pt[:], W_cfc[:, kc, 128 * mm:128 * (mm + 1)],
                            mln_out[b][:, kc, ns], start=(kc == 0),
                            stop=(kc == 1))
                    _gi = nc.scalar.activation(h[:, mm, ns], pt[:], AF.Gelu,
                                               bias=b_cfc_col[:, mm:mm + 1])
                    gelu_insts[b].append(_gi)
            h_sb.append(h)

        from concourse.tile import add_dep_helper
        attention(0)
        cfc(0)
        add_dep_helper(gelu_insts[0][0].ins, exp_insts[0][-1].ins,
                       reason="ACT tables: gelu0 after exp0")
        attention(1)
        add_dep_helper(exp_insts[1][0].ins, gelu_insts[0][-1].ins,
                       reason="ACT tables: exp1 after gelu0")
        cfc(1)
        add_dep_helper(gelu_insts[1][0].ins, exp_insts[1][-1].ins,
                       reason="ACT tables: gelu1 after exp1")


    # ------------- softmax denominators -> column form -------------
    r_col = []
    for b in range(BL):
        rs_dram = dram.tile([S], F32, name=f"rsd{b}", tag=f"rsd{b}")
        nc.gpsimd.dma_start(rs_dram[:], rs_row[b][:])
        rsc = con.tile([128, TCH], F32, name=f"rsc{b}", tag="rsc", bufs=2)
        nc.gpsimd.dma_start(rsc[:], rs_dram.rearrange("(t p) -> p t", p=128))
        rc = con.tile([128, TCH], F32, name=f"rc{b}", tag="rc", bufs=2)
        nc.vector.reciprocal(rc[:], rsc[:])
        r_col.append(rc)

    # ------------- final fused projections (token-major) -------------
    for b in range(BL):
        for t in range(TCH):
            tsl = slice(128 * t, 128 * (t + 1))
            pa = mmtile()
            for kc in range(2):
                nc.tensor.matmul(pa[:, :E], attn_un[b][:, kc, tsl],
                                 W_cproj[:, kc, :],
                                 start=(kc == 0), stop=(kc == 1))
            pm = mmtile()
            for kc in range(2):
                nc.tensor.matmul(pm[:, :E], h_sb[b][:, kc, tsl],
                                 W_cpm[:, kc, :],
                                 start=(kc == 0), stop=(kc == 1))
            o1 = con.tile([128, E], F32, name="o1", tag="o1", bufs=3)
            nc.vector.tensor_tensor(o1[:], pm[:, :E], bfin_bc[:], ALU.add)
            o_sb = con.tile([128, E], F32, name="osb", tag="osb", bufs=3)
            nc.vector.scalar_tensor_tensor(
                o_sb[:], pa[:, :E], r_col[b][:, t:t + 1], o1[:],
                ALU.mult, ALU.add)
            (nc.sync if t % 2 == 0 else nc.scalar).dma_start(
                out_d[b, tsl, :], o_sb[:])

    ctx.close()


def _prep_inputs(p_aln, p_mln, p_cattn, p_cproj, p_cfc, p_cproj_mlp,
                 time_embed, x):
    f32 = np.float32
    bf16 = ml_dtypes.bfloat16
    te = np.asarray(time_embed, f32)
    com = {
        "te_row": te.reshape(1, E).copy(),
        "te_col": te.reshape(2, 128).copy(),
        "te_col_f": te.reshape(2, 128).copy(),
        "ones_row_f": np.ones((1, 128), f32),
    }
    for q, p in (("aln", p_aln), ("mln", p_mln)):
        com[f"{q}_l1w"] = np.asarray(p["lin1_w"], f32).astype(bf16)
        com[f"{q}_l1b_col"] = np.asarray(p["lin1_b"], f32).reshape(2, 128).copy()
        com[f"{q}_l2wT"] = np.asarray(p["lin2_w"], f32).T.copy()
        com[f"{q}_l2b_col"] = np.asarray(p["lin2_b"], f32).reshape(2, 128).copy()
        com[f"{q}_fww"] = np.asarray(p["fw_w"], f32)
        com[f"{q}_fbw"] = np.asarray(p["fb_w"], f32)
        com[f"{q}_fwb_col"] = np.asarray(p["fw_b"], f32).reshape(2, 128).copy()
        com[f"{q}_fbb_col"] = np.asarray(p["fb_b"], f32).reshape(2, 128).copy()
    tl = (("cattn", p_cattn), ("cproj", p_cproj), ("cfc", p_cfc),
          ("cpm", p_cproj_mlp))
    fWs_bf = {}
    for m, p in tl:
        D = TL_OUT[m]
        com[f"{m}_l1w"] = np.asarray(p["lin1_w"], f32)
        com[f"{m}_l1b_col"] = np.asarray(p["lin1_b"], f32).reshape(2, 128).copy()
        com[f"{m}_l2w"] = np.asarray(p["lin2_w"], f32)
        com[f"{m}_l2b_col"] = np.asarray(p["lin2_b"], f32).reshape(2, 128).copy()
        com[f"{m}_fWb"] = np.asarray(p["fW_b"], f32).reshape(E, D) \
            .reshape(2, 128, D).astype(bf16)
        com[f"{m}_fbw"] = np.asarray(p["fb_w"], f32)
        fWs_bf[m] = np.asarray(p["fW_w"], f32).astype(bf16)
    fbb_cattn = np.asarray(p_cattn["fb_b"], f32)
    com["cattn_fbb_qk_col"] = fbb_cattn[:512].reshape(4, 128).copy()
    com["cattn_fbb_v_row"] = fbb_cattn[512:].reshape(1, E).copy()
    com["cproj_fbb_row"] = np.asarray(p_cproj["fb_b"], f32).reshape(1, E).copy()
    com["cfc_fbb_col"] = np.asarray(p_cfc["fb_b"], f32).reshape(2, 128).copy()
    com["cpm_fbb_row"] = np.asarray(p_cproj_mlp["fb_b"], f32).reshape(1, E).copy()

    x_bf = np.asarray(x, f32).astype(bf16)
    in_maps = []
    for c in range(NCORES):
        im = dict(com)
        xs = x_bf[c * BL:(c + 1) * BL]          # [BL, S, E]
        im["x"] = np.ascontiguousarray(
            xs.reshape(BL, S, 2, 128).transpose(0, 2, 3, 1))
        im["cattn_fWs"] = np.ascontiguousarray(
            fWs_bf["cattn"][:, c * C_ATTN:(c + 1) * C_ATTN])
        for m in ("cproj", "cfc", "cpm"):
            im[f"{m}_fWs"] = np.ascontiguousarray(
                fWs_bf[m][:, c * C_SM:(c + 1) * C_SM])
        in_maps.append(im)
    return in_maps


def kernel(p_aln, p_mln, p_cattn, p_cproj, p_cfc, p_cproj_mlp,
           time_embed, x):
    if "nc" not in _cache:
        _cache["nc"] = _build()
    nc = _cache["nc"]
    in_maps = _prep_inputs(p_aln, p_mln, p_cattn, p_cproj, p_cfc,
                           p_cproj_mlp, time_embed, x)
    res = bass_utils.run_bass_kernel_spmd(
        nc, in_maps, core_ids=list(range(NCORES)))
    out = np.concatenate([res.results[c]["out"] for c in range(NCORES)],
                         axis=0)
    return out.astype(np.float32)


# revision 26
# speedup vs baseline: 1.0674x; 1.0089x over previous
"""Trainium2 Bass kernel for nn_Block_6975026889363 (dense transformer block
with hypernetwork-generated weights), SPMD over 8 NeuronCores.

Strategy:
  - Data-parallel over batch (16 batches -> 2 per core).
  - The big hypernet GEMMs (t @ fW_w, 402MB of fW_w in f32) are
    column-sharded across the 8 cores (each core reads 1/8th, host-cast to
    bf16), then the generated W matrices (tiny) are AllGathered on-chip.
  - The trunk stays feature-major ([feature, token]) so weights serve as
    lhsT in natural layout and channel biases are per-partition ACT biases;
    the final projections run token-major so the residual add and output
    DMA need no transpose.
  - Softmax needs no row-max subtraction (scores empirically in [-11,-2.2]);
    scores are computed transposed so exp writes the AV operand directly and
    row sums come from a ones-vector matmul on the TensorEngine.
  - Precision: bf16 for x / LN chain / attention internals / hypernet fW;
    float32r (full speed on PE for free-dim >= 256) for qkv, c_fc, c_proj,
    c_proj_mlp matmuls and the generated weights.
"""
import sys
import types

import numpy as np
import ml_dtypes

# Provide the antenv.axon_hooks shim so trace=True (e.g. via BASS_TRACE=1)
# degrades gracefully / works instead of crashing on import.
try:
    import antenv.axon_hooks  # noqa: F401
except Exception:
    try:
        _mod = types.ModuleType("antenv.axon_hooks")
        _mod._hook = None
        _mod.set_axon_ntff_profile_hook = lambda h: setattr(_mod, "_hook", h)
        _mod.get_axon_ntff_profile_hook = lambda: _mod._hook
        sys.modules["antenv.axon_hooks"] = _mod
        import antenv
        antenv.axon_hooks = _mod
        from trn_agent_boot.trn_boot import _ntff_profile_via_ctypes
        _mod._hook = _ntff_profile_via_ctypes("/opt/axon/libaxon_pjrt.so")
    except Exception:
        pass

import concourse.bass as bass  # noqa: F401
import concourse.bacc as bacc
import concourse.mybir as mybir
import concourse.tile as tile
from concourse import bass_utils

E = 256
B = 16
S = 2048
NCORES = 8
BL = B // NCORES            # batches per core
D3E = 3 * E                 # 768
C_ATTN = E * D3E // NCORES  # fW_w column-shard size for c_attn (24576)
C_SM = E * E // NCORES      # for c_proj / c_fc / c_proj_mlp (8192)
HTILE = 2048                # hypernet fW streaming tile (free dim)
EPS = 1e-5

F32 = mybir.dt.float32
F32R = mybir.dt.float32r
BF16 = mybir.dt.bfloat16
AF = mybir.ActivationFunctionType
ALU = mybir.AluOpType
AX = mybir.AxisListType

LN_MODS = ["aln", "mln"]
TL_MODS = ["cattn", "cproj", "cfc", "cpm"]
TL_OUT = {"cattn": D3E, "cproj": E, "cfc": E, "cpm": E}
TL_COLS = {"cattn": C_ATTN, "cproj": C_SM, "cfc": C_SM, "cpm": C_SM}

_cache = {}


def _build():
    nc = bacc.Bacc("TRN2", target_bir_lowering=False, debug=False,
                   num_devices=NCORES)

    def din(name, shape, dt):
        return nc.dram_tensor(name, shape, dt, kind="ExternalInput").ap()

    ins = {}
    ins["x"] = din("x", [BL, 2, 128, S], BF16)
    ins["te_row"] = din("te_row", [1, E], F32)
    ins["te_col"] = din("te_col", [2, 128], F32)
    ins["te_col_f"] = din("te_col_f", [2, 128], F32)
    ins["ones_row_f"] = din("ones_row_f", [1, 128], F32)
    for q in LN_MODS:
        ins[f"{q}_l1w"] = din(f"{q}_l1w", [E, E], BF16)
        ins[f"{q}_l1b_col"] = din(f"{q}_l1b_col", [2, 128], F32)
        ins[f"{q}_l2wT"] = din(f"{q}_l2wT", [E, E], F32)
        ins[f"{q}_l2b_col"] = din(f"{q}_l2b_col", [2, 128], F32)
        ins[f"{q}_fww"] = din(f"{q}_fww", [E, E], F32)
        ins[f"{q}_fbw"] = din(f"{q}_fbw", [E, E], F32)
        ins[f"{q}_fwb_col"] = din(f"{q}_fwb_col", [2, 128], F32)
        ins[f"{q}_fbb_col"] = din(f"{q}_fbb_col", [2, 128], F32)
    for m in TL_MODS:
        D = TL_OUT[m]
        ins[f"{m}_l1w"] = din(f"{m}_l1w", [E, E], F32)
        ins[f"{m}_l1b_col"] = din(f"{m}_l1b_col", [2, 128], F32)
        ins[f"{m}_l2w"] = din(f"{m}_l2w", [E, E], F32)
        ins[f"{m}_l2b_col"] = din(f"{m}_l2b_col", [2, 128], F32)
        ins[f"{m}_fWs"] = din(f"{m}_fWs", [E, TL_COLS[m]], BF16)
        ins[f"{m}_fWb"] = din(f"{m}_fWb", [2, 128, D], BF16)
        ins[f"{m}_fbw"] = din(f"{m}_fbw", [E, D], F32)
    ins["cattn_fbb_qk_col"] = din("cattn_fbb_qk_col", [4, 128], F32)
    ins["cattn_fbb_v_row"] = din("cattn_fbb_v_row", [1, E], F32)
    ins["cproj_fbb_row"] = din("cproj_fbb_row", [1, E], F32)
    ins["cfc_fbb_col"] = din("cfc_fbb_col", [2, 128], F32)
    ins["cpm_fbb_row"] = din("cpm_fbb_row", [1, E], F32)

    out_d = nc.dram_tensor("out", [BL, S, E], F32, kind="ExternalOutput").ap()

    with tile.TileContext(nc) as tc:
        _emit(nc, tc, ins, out_d)
    nc.compile()
    return nc


def _emit(nc, tc, ins, out_d):
    from contextlib import ExitStack
    NT = S // 512            # 512-wide free chunks per batch (4)
    TCH = S // 128           # token 128-chunks per batch (16)

    ctx = ExitStack()
    B1, B2 = 6, 5
    con = ctx.enter_context(tc.tile_pool(name="con", bufs=1))
    pW = con
    ps_main = ctx.enter_context(tc.tile_pool(name="ps_main", bufs=2,
                                             space="PSUM"))
    dram = ctx.enter_context(tc.tile_pool(name="dram", bufs=1, space="DRAM"))

    def mmtile():
        return ps_main.tile([128, 512], F32, name="mm", tag="mm", bufs=2)

    sb = {}

    def load2(pool, name, dt, d2=E, eng=None):
        t = pool.tile([128, 2, d2], dt, name=name, tag=name)
        (eng or nc.scalar).dma_start(
            t[:], ins[name].rearrange("(kc p) m -> p kc m", p=128))
        return t

    def loadcol(pool, name, w=2, dt=F32, eng=None):
        t = pool.tile([128, w], dt, name=name, tag=name)
        (eng or nc.scalar).dma_start(t[:], ins[name].rearrange("a p -> p a"))
        return t

    def loadrow(pool, name, dt=F32):
        t = pool.tile([1, E], dt, name=name, tag=name)
        nc.scalar.dma_start(t[:], ins[name])
        return t

    # ------------- permanent constants -------------
    sb["te_row"] = con.tile([1, E], F32, name="te_row")
    nc.sync.dma_start(sb["te_row"][:], ins["te_row"])
    sb["te_col"] = loadcol(con, "te_col", dt=F32, eng=nc.sync)
    sb["te_col_f"] = loadcol(con, "te_col_f", dt=F32)
    sb["ones_row_f"] = con.tile([1, 128], F32, name="ones_row_f")
    nc.sync.dma_start(sb["ones_row_f"][:], ins["ones_row_f"])
    for q in LN_MODS:
        sb[f"{q}_l1w"] = load2(con, f"{q}_l1w", BF16)
        sb[f"{q}_l1b_col"] = loadcol(con, f"{q}_l1b_col")
    for m in TL_MODS:
        sb[f"{m}_l1b_col"] = loadcol(con, f"{m}_l1b_col")
        sb[f"{m}_l2b_col"] = loadcol(con, f"{m}_l2b_col")
        sb[f"{m}_fWb"] = con.tile([128, 2, TL_OUT[m]], BF16, name=f"{m}_fWb",
                                  tag=f"{m}_fWb")
        nc.sync.dma_start(sb[f"{m}_fWb"][:],
                          ins[f"{m}_fWb"].rearrange("a p d -> p a d"))
    sb["cattn_fbb_qk_col"] = loadcol(con, "cattn_fbb_qk_col", w=4)
    sb["cfc_fbb_col"] = loadcol(con, "cfc_fbb_col")
    sb["cattn_fbb_v_row"] = loadrow(con, "cattn_fbb_v_row", dt=F32)
    sb["cproj_fbb_row"] = loadrow(con, "cproj_fbb_row", dt=F32)
    sb["cpm_fbb_row"] = loadrow(con, "cpm_fbb_row", dt=F32)

    ones_col_bf = con.tile([128, 1], BF16, name="ones_col_bf")
    nc.vector.memset(ones_col_bf[:], 1.0)

    # ------------- early-only weights (scoped; closes before 'big' opens) ---
    _early_cm = tc.tile_pool(name="early", bufs=1)
    ep = _early_cm.__enter__()
    for m in TL_MODS:
        sb[f"{m}_l1w"] = load2(ep, f"{m}_l1w", F32, eng=nc.sync)
        sb[f"{m}_l2w"] = load2(ep, f"{m}_l2w", F32, eng=nc.sync)
    for q in LN_MODS:
        sb[f"{q}_l2wT"] = load2(ep, f"{q}_l2wT", F32)
        sb[f"{q}_l2b_col"] = loadcol(ep, f"{q}_l2b_col", dt=F32)
        sb[f"{q}_fww"] = load2(ep, f"{q}_fww", F32)
        sb[f"{q}_fbw"] = load2(ep, f"{q}_fbw", F32)
        sb[f"{q}_fwb_col"] = loadcol(ep, f"{q}_fwb_col")
        sb[f"{q}_fbb_col"] = loadcol(ep, f"{q}_fbb_col")
    for m in TL_MODS:
        sb[f"{m}_fbw"] = load2(ep, f"{m}_fbw", F32, d2=TL_OUT[m])

    # ------------- time-embed stats -> norm -------------
    st = ep.tile([1, 8], F32, name="st")
    nc.vector.reduce_sum(st[:, 0:1], sb["te_row"][:], axis=AX.X)
    nc.vector.tensor_scalar_mul(st[:, 1:2], st[:, 0:1], 1.0 / E)   # mean
    cen = ep.tile([1, E], F32, name="cen")
    nc.vector.tensor_scalar(cen[:], sb["te_row"][:], st[:, 1:2], None,
                            ALU.subtract)
    sq = ep.tile([1, E], F32, name="sq")
    nc.vector.tensor_tensor(sq[:], cen[:], cen[:], ALU.mult)
    nc.vector.reduce_sum(st[:, 2:3], sq[:], axis=AX.X)
    nc.vector.tensor_scalar(st[:, 3:4], st[:, 2:3], 1.0 / E, EPS,
                            ALU.mult, ALU.add)                      # var+eps
    # sqrt seed + one Newton step, then rstd = 1/sqrt
    nc.scalar.activation(st[:, 4:5], st[:, 3:4], AF.Sqrt)
    nc.vector.reciprocal(st[:, 5:6], st[:, 4:5])
    nc.vector.tensor_tensor(st[:, 6:7], st[:, 3:4], st[:, 5:6], ALU.mult)
    nc.vector.tensor_tensor(st[:, 6:7], st[:, 6:7], st[:, 4:5], ALU.add)
    nc.vector.tensor_scalar_mul(st[:, 6:7], st[:, 6:7], 0.5)        # sqrt(v)
    nc.vector.reciprocal(st[:, 7:8], st[:, 6:7])                    # rstd
    norm_row = ep.tile([1, E], F32, name="norm_row")
    nc.vector.tensor_scalar(norm_row[:], cen[:], st[:, 7:8], None, ALU.mult)

    # broadcast mean/rstd across partitions via K=1 matmuls, then norm_col
    mr_sb = ep.tile([128, 2], F32, name="mr_sb")
    pt = mmtile()
    nc.tensor.matmul(pt[:, 0:1], sb["ones_row_f"][:], st[:, 1:2],
                     start=True, stop=True)
    nc.tensor.matmul(pt[:, 1:2], sb["ones_row_f"][:], st[:, 7:8],
                     start=True, stop=True)
    nc.vector.tensor_copy(mr_sb[:], pt[:, 0:2])
    norm_col = ep.tile([128, 2], F32, name="norm_col")
    nc.vector.tensor_scalar(norm_col[:], sb["te_col_f"][:], mr_sb[:, 0:1],
                            None, ALU.subtract)
    nc.vector.tensor_scalar(norm_col[:], norm_col[:], mr_sb[:, 1:2],
                            None, ALU.mult)
    # norm broadcast across partitions (for Wc prep)
    norm_bc = ep.tile([128, E], F32, name="norm_bc")
    pt = mmtile()
    nc.tensor.matmul(pt[:, :E], sb["ones_row_f"][:], norm_row[:],
                     start=True, stop=True)
    nc.vector.tensor_copy(norm_bc[:], pt[:, :E])

    # ------------- t vectors (4 TL modules) -------------
    t_col, t_col_bf = {}, {}
    for m in TL_MODS:
        h_t = ep.tile([128, 2], F32, name=f"ht_{m}")
        for mm in range(2):
            pt = mmtile()
            for kc in range(2):
                nc.tensor.matmul(pt[:, 0:1],
                                 sb[f"{m}_l1w"][:, kc, 128 * mm:128 * (mm + 1)],
                                 sb["te_col"][:, kc:kc + 1],
                                 start=(kc == 0), stop=(kc == 1))
            nc.scalar.activation(h_t[:, mm:mm + 1], pt[:, 0:1], AF.Silu,
                                 bias=sb[f"{m}_l1b_col"][:, mm:mm + 1])
        tcl = con.tile([128, 2], F32, name=f"t_{m}")
        for mm in range(2):
            pt = mmtile()
            for kc in range(2):
                nc.tensor.matmul(pt[:, 0:1],
                                 sb[f"{m}_l2w"][:, kc, 128 * mm:128 * (mm + 1)],
                                 h_t[:, kc:kc + 1],
                                 start=(kc == 0), stop=(kc == 1))
            nc.vector.tensor_scalar(tcl[:, mm:mm + 1], pt[:, 0:1],
                                    sb[f"{m}_l2b_col"][:, mm:mm + 1],
                                    None, ALU.add)
        t_col[m] = tcl
        tb = con.tile([128, 2], BF16, name=f"tbf_{m}")
        nc.vector.tensor_copy(tb[:], tcl[:])
        t_col_bf[m] = tb

    # ------------- fused LN weights (W2c bf16, b2c col) -------------
    W2c, b2c = {}, {}
    for q in LN_MODS:
        wc = ep.tile([128, 2, E], F32, name=f"wc_{q}")
        for kc in range(2):
            nc.vector.tensor_tensor(wc[:, kc, :], sb[f"{q}_fww"][:, kc, :],
                                    norm_bc[:], ALU.mult)
            nc.vector.tensor_tensor(wc[:, kc, :], wc[:, kc, :],
                                    sb[f"{q}_fbw"][:, kc, :], ALU.add)
        w2 = con.tile([128, 2, E], BF16, name=f"w2c_{q}")
        for mm in range(2):
            pt = mmtile()
            for kc in range(2):
                nc.tensor.matmul(pt[:, :E],
                                 sb[f"{q}_l2wT"][:, kc, 128 * mm:128 * (mm + 1)],
                                 wc[:, kc, :], start=(kc == 0), stop=(kc == 1))
            nc.vector.tensor_copy(w2[:, mm, :], pt[:, :E])
        W2c[q] = w2
        bc = ep.tile([128, 2], F32, name=f"bc_{q}")
        nc.vector.tensor_scalar(bc[:], sb[f"{q}_fwb_col"][:], 1.0, None,
                                ALU.add)
        nc.vector.tensor_tensor(bc[:], bc[:], norm_col[:], ALU.mult)
        nc.vector.tensor_tensor(bc[:], bc[:], sb[f"{q}_fbb_col"][:], ALU.add)
        b2 = con.tile([128, 2], F32, name=f"b2c_{q}")
        for mm in range(2):
            pt = mmtile()
            for kc in range(2):
                nc.tensor.matmul(pt[:, 0:1],
                                 wc[:, kc, 128 * mm:128 * (mm + 1)],
                                 sb[f"{q}_l2b_col"][:, kc:kc + 1],
                                 start=(kc == 0), stop=(kc == 1))
            nc.vector.tensor_tensor(b2[:, mm:mm + 1], pt[:, 0:1],
                                    bc[:, mm:mm + 1], ALU.add)
        b2c[q] = b2

    # ------------- hypernet-generated biases -------------
    b_qk_col = con.tile([128, 4], F32, name="b_qk_col")
    for mm in range(4):
        pt = mmtile()
        for kc in range(2):
            nc.tensor.matmul(pt[:, 0:1],
                             sb["cattn_fbw"][:, kc, 128 * mm:128 * (mm + 1)],
                             t_col["cattn"][:, kc:kc + 1],
                             start=(kc == 0), stop=(kc == 1))
        nc.vector.tensor_tensor(b_qk_col[:, mm:mm + 1], pt[:, 0:1],
                                sb["cattn_fbb_qk_col"][:, mm:mm + 1], ALU.add)
    b_cfc_col = con.tile([128, 2], F32, name="b_cfc_col")
    for mm in range(2):
        pt = mmtile()
        for kc in range(2):
            nc.tensor.matmul(pt[:, 0:1],
                             sb["cfc_fbw"][:, kc, 128 * mm:128 * (mm + 1)],
                             t_col["cfc"][:, kc:kc + 1],
                             start=(kc == 0), stop=(kc == 1))
        nc.vector.tensor_tensor(b_cfc_col[:, mm:mm + 1], pt[:, 0:1],
                                sb["cfc_fbb_col"][:, mm:mm + 1], ALU.add)

    def brow(mod, cols, fbb_name, name):
        r = con.tile([1, E], F32, name=name)
        pt = mmtile()
        for kc in range(2):
            nc.tensor.matmul(pt[0:1, :E], t_col[mod][:, kc:kc + 1],
                             sb[f"{mod}_fbw"][:, kc, cols],
                             start=(kc == 0), stop=(kc == 1))
        nc.vector.tensor_tensor(r[:], pt[0:1, :E], sb[fbb_name][:], ALU.add)
        return r

    b_v_row = brow("cattn", slice(512, 768), "cattn_fbb_v_row", "b_v_row")
    b_cproj_row = brow("cproj", slice(0, E), "cproj_fbb_row", "b_cproj_row")
    b_cpm_row = brow("cpm", slice(0, E), "cpm_fbb_row", "b_cpm_row")
    b_fin_row = con.tile([1, E], F32, name="b_fin_row")
    nc.vector.tensor_tensor(b_fin_row[:], b_cproj_row[:], b_cpm_row[:],
                            ALU.add)
    # broadcast rows across partitions once (K=1 fp32 matmuls kept out of the
    # hot bf16 streams)
    bv_bc = con.tile([128, E], F32, name="bv_bc")
    pt = mmtile()
    nc.tensor.matmul(pt[:, :E], sb["ones_row_f"][:], b_v_row[:],
                     start=True, stop=True)
    nc.vector.tensor_copy(bv_bc[:], pt[:, :E])
    bfin_bc = con.tile([128, E], F32, name="bfin_bc")
    pt = mmtile()
    nc.tensor.matmul(pt[:, :E], sb["ones_row_f"][:], b_fin_row[:],
                     start=True, stop=True)
    nc.vector.tensor_copy(bfin_bc[:], pt[:, :E])

    # ------------- close early pool; open the big activation pool -------------
    _early_cm.__exit__(None, None, None)
    big = ctx.enter_context(tc.tile_pool(name="big", bufs=1))

    # ------------- x loads (DMA transpose, bf16) -------------
    xT = []
    for b in range(BL):
        t = big.tile([128, 2, S], BF16, name=f"xT{b}", tag="b1", bufs=B1)
        nc.sync.dma_start(t[:], ins["x"][b].rearrange("kc p s -> p kc s"))
        xT.append(t)


    # ------------- hypernet fW slices + AllGather -------------
    warm_in = dram.tile([16], F32, name="warm_in")
    warm_out = dram.tile([NCORES, 16], F32, name="warm_out",
                         addr_space="Shared")
    warm_sb = con.tile([1, 16], F32, name="warm_sb")
    nc.vector.memset(warm_sb[:], 0.0)
    nc.gpsimd.dma_start(warm_in[:], warm_sb[:])
    nc.gpsimd.collective_compute(
        "AllGather", ALU.bypass,
        replica_groups=[list(range(NCORES))],
        ins=[warm_in.opt()], outs=[warm_out.opt()])

    w_slice_cattn = dram.tile([C_ATTN], F32R, name="wsl_cattn")
    w_gather_cattn = dram.tile([NCORES, C_ATTN], F32R, name="wg_cattn",
                               addr_space="Shared")
    w_slice_rest = dram.tile([3, C_SM], F32R, name="wsl_rest")
    w_gather_rest = dram.tile([NCORES, 3, C_SM], F32R, name="wg_rest",
                              addr_space="Shared")

    with tc.tile_pool(name="ps_hy", bufs=2, space="PSUM") as ps_hy:

        def hyper_mod(m, dst_ap):
            cols = TL_COLS[m]
            fws = ins[f"{m}_fWs"].rearrange("(kc p) c -> p kc c", p=128)
            for nt in range(cols // HTILE):
                ft = big.tile([128, 2, HTILE], BF16, name="hyft", tag="b1",
                              bufs=B1)
                if m == "cattn":
                    eng = nc.sync
                else:
                    eng = nc.sync if nt % 2 == 0 else nc.scalar
                eng.dma_start(ft[:], fws[:, :, nt * HTILE:(nt + 1) * HTILE])
                # 4 col-tiled M=1 matmuls land the four 512-col sub-chunks on
                # partitions 0/32/64/96 of one PSUM bank -> one 4-row evac.
                hp = ps_hy.tile([128, 512], F32, name="hy", tag="hy", bufs=3)
                for j in range(4):
                    for kc in range(2):
                        nc.tensor.matmul(
                            hp[32 * j:32 * j + 1, :],
                            t_col_bf[m][:, kc:kc + 1],
                            ft[:, kc, 512 * j:512 * (j + 1)],
                            start=(kc == 0), stop=(kc == 1),
                            tile_position=(0, 32 * j))
                hs = con.tile([128, 512], F32R, name="hys", tag="hys",
                              bufs=4)
                for j in range(4):
                    srow = hp[32 * j:32 * j + 1, :]
                    drow = hs[32 * j:32 * j + 1, :]
                    if (nt + j) % 2 == 0:
                        nc.vector.tensor_copy(drow, srow)
                    else:
                        nc.scalar.activation(drow, srow, AF.Copy)
                off = nt * HTILE
                nc.scalar.dma_start(dst_ap[off:off + HTILE], hs[0:128:32, :])

        hyper_mod("cattn", w_slice_cattn)
        nc.gpsimd.collective_compute(
            "AllGather", ALU.bypass,
            replica_groups=[list(range(NCORES))],
            ins=[w_slice_cattn.opt()], outs=[w_gather_cattn.opt()])
        for mi, m in enumerate(["cproj", "cfc", "cpm"]):
            hyper_mod(m, w_slice_rest[mi])
        nc.gpsimd.collective_compute(
            "AllGather", ALU.bypass,
            replica_groups=[list(range(NCORES))],
            ins=[w_slice_rest.opt()], outs=[w_gather_rest.opt()])

    # ------------- aln / mln (both batches) -------------
    def temporal_ln(q, b, out_tile):
        h1 = big.tile([128, 2, S], BF16, name=f"h1_{q}{b}", tag="b1", bufs=B1)
        for mm in range(2):
            for n in range(NT):
                ns = slice(512 * n, 512 * (n + 1))
                pt = mmtile()
                for kc in range(2):
                    nc.tensor.matmul(
                        pt[:], sb[f"{q}_l1w"][:, kc, 128 * mm:128 * (mm + 1)],
                        xT[b][:, kc, ns], start=(kc == 0), stop=(kc == 1))
                nc.scalar.activation(h1[:, mm, ns], pt[:], AF.Silu,
                                     bias=sb[f"{q}_l1b_col"][:, mm:mm + 1])
        for mm in range(2):
            for n in range(NT):
                ns = slice(512 * n, 512 * (n + 1))
                pt = mmtile()
                for kc in range(2):
                    nc.tensor.matmul(
                        pt[:], W2c[q][:, kc, 128 * mm:128 * (mm + 1)],
                        h1[:, kc, ns], start=(kc == 0), stop=(kc == 1))
                nc.vector.tensor_scalar(out_tile[:, mm, ns], pt[:],
                                        b2c[q][:, mm:mm + 1], None, ALU.add)

    aln_out, mln_out = [], []
    for b in range(BL):
        t = big.tile([128, 2, S], F32R, name=f"alno{b}", tag="b2", bufs=B2)
        temporal_ln("aln", b, t)
        aln_out.append(t)
    for b in range(BL):
        t = big.tile([128, 2, S], F32R, name=f"mlno{b}", tag="b2", bufs=B2)
        temporal_ln("mln", b, t)
        mln_out.append(t)

    # ------------- assemble gathered W matrices -------------
    def assemble(name, gath_ap, D, fWb):
        w = pW.tile([128, 2, D], F32R, name=name, tag=name, bufs=1)
        for c in range(NCORES):
            nc.scalar.dma_start(
                w[32 * (c % 4):32 * (c % 4) + 32, c // 4, :],
                gath_ap[c].rearrange("(r o) -> r o", o=D))
        nc.vector.tensor_tensor(w[:], w[:], fWb[:], ALU.add)
        return w

    W_cattn = assemble("W_cattn", w_gather_cattn, D3E, sb["cattn_fWb"])
    W_cproj = assemble("W_cproj", w_gather_rest[:, 0], E, sb["cproj_fWb"])
    W_cfc = assemble("W_cfc", w_gather_rest[:, 1], E, sb["cfc_fWb"])
    W_cpm = assemble("W_cpm", w_gather_rest[:, 2], E, sb["cpm_fWb"])

    # ------------- qkv (Q^T,K^T feature-major; V token-major) -------------
    q_sb, k_sb, v_sb = [], [], []
    for b in range(BL):
        q = big.tile([128, 2, S], BF16, name=f"q{b}", tag="b1", bufs=B1)
        k = big.tile([128, 2, S], BF16, name=f"k{b}", tag="b1", bufs=B1)
        for mm in range(4):
            dst = q[:, mm, :] if mm < 2 else k[:, mm - 2, :]
            for n in range(NT):
                ns = slice(512 * n, 512 * (n + 1))
                pt = mmtile()
                for kc in range(2):
                    nc.tensor.matmul(
                        pt[:], W_cattn[:, kc, 128 * mm:128 * (mm + 1)],
                        aln_out[b][:, kc, ns], start=(kc == 0), stop=(kc == 1))
                nc.vector.tensor_scalar(dst[:, ns], pt[:],
                                        b_qk_col[:, mm:mm + 1], None, ALU.add)
        q_sb.append(q)
        k_sb.append(k)
        v = big.tile([128, TCH, E], BF16, name=f"v{b}", tag="b1", bufs=B1)
        for t in range(TCH):
            pt = mmtile()
            for kc in range(2):
                nc.tensor.matmul(
                    pt[:, :E], aln_out[b][:, kc, 128 * t:128 * (t + 1)],
                    W_cattn[:, kc, 512:768], start=(kc == 0), stop=(kc == 1))
            nc.vector.tensor_tensor(v[:, t, :], pt[:, :E], bv_bc[:], ALU.add)
        v_sb.append(v)

    # ------------- attention (both batches) -------------
    exp_insts = {0: [], 1: []}
    gelu_insts = {0: [], 1: []}
    attn_un = []
    rs_row = []
    h_sb = []
    with tc.tile_pool(name="ps_sc", bufs=2, space="PSUM") as ps_sc, \
         tc.tile_pool(name="ps_av", bufs=2, space="PSUM") as ps_av, \
         tc.tile_pool(name="ps_rs", bufs=1, space="PSUM") as ps_rs:

        def attention(b):
            attn_un.append(big.tile([128, 2, S], F32R, name=f"au{b}",
                                    tag="b2", bufs=B2))
            rs_row.append(con.tile([1, S], F32, name=f"rs{b}", tag="rsr",
                                   bufs=2))
            for sc in range(NT):
                s1 = slice(512 * sc, 512 * (sc + 1))
                av = [ps_av.tile([128, 512], F32, name="av", tag="av", bufs=2)
                      for _ in range(2)]
                rs = ps_rs.tile([1, 512], F32, name="rsp", tag="rsp", bufs=1)
                for s2 in range(TCH):
                    sp = ps_sc.tile([128, 512], F32, name="sc", tag="sc",
                                    bufs=3)
                    for kc in range(2):
                        nc.tensor.matmul(
                            sp[:], k_sb[b][:, kc, 128 * s2:128 * (s2 + 1)],
                            q_sb[b][:, kc, s1], start=(kc == 0),
                            stop=(kc == 1))
                    mt = con.tile([128, 512], BF16, name="mt", tag="mt",
                                  bufs=4)
                    _ei = nc.scalar.activation(mt[:], sp[:], AF.Exp,
                                               scale=1.0 / 16.0)
                    exp_insts[b].append(_ei)
                    nc.tensor.matmul(rs[:], ones_col_bf[:], mt[:],
                                     start=(s2 == 0), stop=(s2 == TCH - 1))
                    for e in range(2):
                        nc.tensor.matmul(
                            av[e][:], v_sb[b][:, s2, 128 * e:128 * (e + 1)],
                            mt[:], start=(s2 == 0), stop=(s2 == TCH - 1))
                nc.vector.tensor_copy(rs_row[b][:, s1], rs[:])
                for e in range(2):
                    nc.vector.tensor_copy(attn_un[b][:, e, s1], av[e][:])

        def cfc(b):
            h = big.tile([128, 2, S], F32R, name=f"h{b}", tag="b2", bufs=B2)
            for mm in range(2):
                for n in range(NT):
                    ns = slice(512 * n, 512 * (n + 1))
                    pt = mmtile()
                    for kc in range(2):
                        nc.tensor.matmul(
                            pt[:], W_cfc[:, kc, 128 * mm:128 * (mm + 1)],
                            mln_out[b][:, kc, ns], start=(kc == 0),
                            stop=(kc == 1))
                    _gi = nc.scalar.activation(h[:, mm, ns], pt[:], AF.Gelu,
                                               bias=b_cfc_col[:, mm:mm + 1])
                    gelu_insts[b].append(_gi)
            h_sb.append(h)

        from concourse.tile import add_dep_helper
        attention(0)
        cfc(0)
        add_dep_helper(gelu_insts[0][0].ins, exp_insts[0][-1].ins,
                       reason="ACT tables: gelu0 after exp0")
        attention(1)
        add_dep_helper(exp_insts[1][0].ins, gelu_insts[0][-1].ins,
                       reason="ACT tables: exp1 after gelu0")
        cfc(1)
        add_dep_helper(gelu_insts[1][0].ins, exp_insts[1][-1].ins,
                       reason="ACT tables: gelu1 after exp1")


    # ------------- softmax denominators -> column form -------------
    r_col = []
    for b in range(BL):
        rs_dram = dram.tile([S], F32, name=f"rsd{b}", tag=f"rsd{b}")
        nc.gpsimd.dma_start(rs_dram[:], rs_row[b][:])
        rsc = con.tile([128, TCH], F32, name=f"rsc{b}", tag="rsc", bufs=2)
        nc.gpsimd.dma_start(rsc[:], rs_dram.rearrange("(t p) -> p t", p=128))
        rc = con.tile([128, TCH], F32, name=f"rc{b}", tag="rc", bufs=2)
        nc.vector.reciprocal(rc[:], rsc[:])
        r_col.append(rc)

    # ------------- final fused projections (token-major) -------------
    for b in range(BL):
        for t in range(TCH):
            tsl = slice(128 * t, 128 * (t + 1))
            pa = mmtile()
            for kc in range(2):
                nc.tensor.matmul(pa[:, :E], attn_un[b][:, kc, tsl],
                                 W_cproj[:, kc, :],
                                 start=(kc == 0), stop=(kc == 1))
            pm = mmtile()
            for kc in range(2):
                nc.tensor.matmul(pm[:, :E], h_sb[b][:, kc, tsl],
                                 W_cpm[:, kc, :],
                                 start=(kc == 0), stop=(kc == 1))
            o1 = con.tile([128, E], F32, name="o1", tag="o1", bufs=3)
            nc.vector.tensor_tensor(o1[:], pm[:, :E], bfin_bc[:], ALU.add)
            o_sb = con.tile([128, E], F32, name="osb", tag="osb", bufs=3)
            nc.vector.scalar_tensor_tensor(
                o_sb[:], pa[:, :E], r_col[b][:, t:t + 1], o1[:],
                ALU.mult, ALU.add)
            (nc.sync if t % 2 == 0 else nc.scalar).dma_start(
                out_d[b, tsl, :], o_sb[:])

    ctx.close()


def _prep_inputs(p_aln, p_mln, p_cattn, p_cproj, p_cfc, p_cproj_mlp,
                 time_embed, x):
    f32 = np.float32
    bf16 = ml_dtypes.bfloat16
    te = np.asarray(time_embed, f32)
    com = {
        "te_row": te.reshape(1, E).copy(),
        "te_col": te.reshape(2, 128).copy(),
        "te_col_f": te.reshape(2, 128).copy(),
        "ones_row_f": np.ones((1, 128), f32),
    }
    for q, p in (("aln", p_aln), ("mln", p_mln)):
        com[f"{q}_l1w"] = np.asarray(p["lin1_w"], f32).astype(bf16)
        com[f"{q}_l1b_col"] = np.asarray(p["lin1_b"], f32).reshape(2, 128).copy()
        com[f"{q}_l2wT"] = np.asarray(p["lin2_w"], f32).T.copy()
        com[f"{q}_l2b_col"] = np.asarray(p["lin2_b"], f32).reshape(2, 128).copy()
        com[f"{q}_fww"] = np.asarray(p["fw_w"], f32)
        com[f"{q}_fbw"] = np.asarray(p["fb_w"], f32)
        com[f"{q}_fwb_col"] = np.asarray(p["fw_b"], f32).reshape(2, 128).copy()
        com[f"{q}_fbb_col"] = np.asarray(p["fb_b"], f32).reshape(2, 128).copy()
    tl = (("cattn", p_cattn), ("cproj", p_cproj), ("cfc", p_cfc),
          ("cpm", p_cproj_mlp))
    fWs_bf = {}
    for m, p in tl:
        D = TL_OUT[m]
        com[f"{m}_l1w"] = np.asarray(p["lin1_w"], f32)
        com[f"{m}_l1b_col"] = np.asarray(p["lin1_b"], f32).reshape(2, 128).copy()
        com[f"{m}_l2w"] = np.asarray(p["lin2_w"], f32)
        com[f"{m}_l2b_col"] = np.asarray(p["lin2_b"], f32).reshape(2, 128).copy()
        com[f"{m}_fWb"] = np.asarray(p["fW_b"], f32).reshape(E, D) \
            .reshape(2, 128, D).astype(bf16)
        com[f"{m}_fbw"] = np.asarray(p["fb_w"], f32)
        fWs_bf[m] = np.asarray(p["fW_w"], f32).astype(bf16)
    fbb_cattn = np.asarray(p_cattn["fb_b"], f32)
    com["cattn_fbb_qk_col"] = fbb_cattn[:512].reshape(4, 128).copy()
    com["cattn_fbb_v_row"] = fbb_cattn[512:].reshape(1, E).copy()
    com["cproj_fbb_row"] = np.asarray(p_cproj["fb_b"], f32).reshape(1, E).copy()
    com["cfc_fbb_col"] = np.asarray(p_cfc["fb_b"], f32).reshape(2, 128).copy()
    com["cpm_fbb_row"] = np.asarray(p_cproj_mlp["fb_b"], f32).reshape(1, E).copy()

    x_bf = np.asarray(x, f32).astype(bf16)
    in_maps = []
    for c in range(NCORES):
        im = dict(com)
        xs = x_bf[c * BL:(c + 1) * BL]          # [BL, S, E]
        im["x"] = np.ascontiguousarray(
            xs.reshape(BL, S, 2, 128).transpose(0, 2, 3, 1))
        im["cattn_fWs"] = np.ascontiguousarray(
            fWs_bf["cattn"][:, c * C_ATTN:(c + 1) * C_ATTN])
        for m in ("cproj", "cfc", "cpm"):
            im[f"{m}_fWs"] = np.ascontiguousarray(
                fWs_bf[m][:, c * C_SM:(c + 1) * C_SM])
        in_maps.append(im)
    return in_maps


def kernel(p_aln, p_mln, p_cattn, p_cproj, p_cfc, p_cproj_mlp,
           time_embed, x):
    if "nc" not in _cache:
        _cache["nc"] = _build()
    nc = _cache["nc"]
    in_maps = _prep_inputs(p_aln, p_mln, p_cattn, p_cproj, p_cfc,
                           p_cproj_mlp, time_embed, x)
    res = bass_utils.run_bass_kernel_spmd(
        nc, in_maps, core_ids=list(range(NCORES)))
    out = np.concatenate([res.results[c]["out"] for c in range(NCORES)],
                         axis=0)
    return out.astype(np.float32)
